# revision 18
# baseline (speedup 1.0000x reference)
"""Self-contained Trainium2 Bass kernel for the 2-layer GCN problem.

kernel(**inputs) -> np.ndarray [100000, 5] float32 (log-softmax outputs).

Strategy: destination-node slabs of 12500 across the 8 NeuronCores.
x arrives host-pre-transposed to feature-major bf16 (F, ND) so conv1
loads straight into the PE strip layout; activations ship through the
AllGather as bf16 and are expanded to the fp32 gather table on the DVE
via a half-slab staging tile.  Graph aggregation per source-slab chunk:
GPSIMD ap_gather from the table (replicated over the 8 Q7 core groups),
chained fp32 running cumsum on the DVE, one boundary ap_gather at
per-node last-edge positions, adjacent diff.  Degrees are derived
on-device from the dst-segment position tensors.  The host does index
preprocessing (sorting/partitioning edge_index per the sharding
contract), the x transpose/cast, and input sharding/unsharding.
"""
import os, sys
sys.path.insert(0, "/opt/trn_rl_repo")

_HOOK_OK = False


def _install_ntff_hook():
    global _HOOK_OK
    try:
        import types
        import antenv
        mod = types.ModuleType("antenv.axon_hooks")
        _h = {}
        mod.set_axon_ntff_profile_hook = lambda h: _h.__setitem__("h", h)
        mod.get_axon_ntff_profile_hook = lambda: _h.get("h")
        sys.modules["antenv.axon_hooks"] = mod
        antenv.axon_hooks = mod
        from trn_agent_boot.trn_boot import _ntff_profile_via_ctypes
        mod.set_axon_ntff_profile_hook(
            _ntff_profile_via_ctypes("/opt/axon/libaxon_pjrt.so"))
        from concourse import bass_utils as _bu
        _bu.upload_artifacts = lambda tmpdir: tmpdir
        _HOOK_OK = True
    except Exception:
        _HOOK_OK = False


_install_ntff_hook()

"""builder"""
import numpy as np
from contextlib import ExitStack

import concourse.bacc as bacc
import concourse.bass as bass
import concourse.mybir as mybir
import concourse.tile as tile
from concourse import library_config

F32 = mybir.dt.float32
BF16 = mybir.dt.bfloat16
I16 = mybir.dt.int16
AOT = mybir.AluOpType
AFT = mybir.ActivationFunctionType

NC = 8
QC = 8
TILE_E = 2560


def pad_to(x, m):
    return (x + m - 1) // m * m


class Cfg:
    def __init__(self, n_nodes, f_in=512, h=16, c=5):
        self.N = n_nodes
        self.SLAB = -(-n_nodes // NC)
        self.SL = pad_to(-(-self.SLAB // QC), 128)
        self.ND = self.SL * QC
        self.SLQ = -(-self.SLAB // QC)         # balanced agg-slice size
        self.F_IN, self.H, self.C = f_in, h, c
        self.KCH = f_in // 128
        nwin = None
        for cand in (512, 448, 416, 384, 320, 256, 128):
            if self.SL % cand == 0:
                nwin = cand
                break
        assert nwin, self.SL
        self.NWIN = nwin


def host_prep(edge_index, cfg):
    src = np.asarray(edge_index[0], np.int64)
    dst = np.asarray(edge_index[1], np.int64)
    cores = []
    for k in range(NC):
        d_lo = k * cfg.SLAB
        d_hi = min((k + 1) * cfg.SLAB, cfg.N)
        sel = (dst >= d_lo) & (dst < d_hi)
        s, d = src[sel], dst[sel] - d_lo
        chunks = []
        for c in range(NC):
            lo = c * cfg.SLAB
            hi = min(lo + cfg.SLAB, cfg.N)
            m = (s >= lo) & (s < hi)
            sc, dc = s[m] - lo, d[m]
            o = np.argsort(dc, kind="stable")
            chunks.append((sc[o].astype(np.int64), dc[o].astype(np.int64)))
        cores.append(chunks)
    meta = dict(ntot=[], widths=[])
    for c in range(NC):
        maxe = 1
        for k in range(NC):
            sc, dc = cores[k][c]
            if len(dc):
                maxe = max(maxe, int(np.bincount(
                    np.minimum(dc // cfg.SLQ, QC - 1), minlength=QC).max()))
        ntot = pad_to(maxe + 1, 16)
        # split into ~equal %16 tiles of at most TILE_E
        nt = -(-ntot // TILE_E)
        widths = []
        rem = ntot
        for t in range(nt, 0, -1):
            w = pad_to(-(-rem // t), 16) if t > 1 else rem
            w = min(w, rem)
            widths.append(w)
            rem -= w
        assert sum(widths) == ntot and all(x % 16 == 0 for x in widths)
        meta["ntot"].append(ntot)
        meta["widths"].append(widths)
    per_core = []
    for k in range(NC):
        ins = {}
        for c in range(NC):
            ntot = meta["ntot"][c]
            sc, dc = cores[k][c]
            idxw = np.zeros((128, ntot // 16), np.int16)
            posw = np.zeros((128, cfg.SL // 16), np.int16)
            posp = np.zeros((128, cfg.SL), np.int16)
            for q in range(QC):
                a = q * cfg.SLQ
                b = min((q + 1) * cfg.SLQ, cfg.SLAB) if q < QC - 1 else cfg.SLAB
                mm = (dc >= a) & (dc < b)
                sq = sc[mm]
                dq = dc[mm] - a
                idx = np.full(ntot, cfg.SLAB, np.int64)
                idx[1:1 + len(sq)] = sq
                pos = np.cumsum(np.bincount(dq, minlength=cfg.SL))
                idxw[16 * q:16 * (q + 1)] = idx.reshape(-1, 16).T.astype(np.int16)
                posw[16 * q:16 * (q + 1)] = pos.reshape(-1, 16).T.astype(np.int16)
                posp[16 * q:16 * (q + 1)] = pos[None, :].astype(np.int16)
            ins[f"idx{c}"] = idxw
            ins[f"posw{c}"] = posw
            ins[f"posp{c}"] = posp
        per_core.append(ins)
    return meta, per_core


def build(cfg, meta):
    nc = bacc.Bacc("TRN2", target_bir_lowering=False)
    F, H, C = cfg.F_IN, cfg.H, cfg.C
    ND, SL, SLAB, NWIN = cfg.ND, cfg.SL, cfg.SLAB, cfg.NWIN
    SLQ = cfg.SLQ
    def ssize(q):
        return (SLAB - q * SLQ) if q == QC - 1 else min(SLQ, SLAB - q * SLQ)
    TBLN = SLAB + 16
    NTMX = max(meta["ntot"])

    x_t = nc.dram_tensor("x", (F, ND), BF16, kind="ExternalInput")
    w_t = {}
    for nm, sh in [("W1", (F, H)), ("b1", (H,)), ("W2", (H, C)), ("b2", (C,)),
                   ("W3", (C, 32)), ("b3", (32,)), ("W4", (32, C)), ("b4", (C,))]:
        w_t[nm] = nc.dram_tensor(nm, sh, F32, kind="ExternalInput")
    idx_t, posw_t, posp_t = [], [], []
    for c in range(NC):
        idx_t.append(nc.dram_tensor(f"idx{c}", (128, meta["ntot"][c] // 16), I16,
                                    kind="ExternalInput"))
        posw_t.append(nc.dram_tensor(f"posw{c}", (128, SL // 16), I16,
                                     kind="ExternalInput"))
        posp_t.append(nc.dram_tensor(f"posp{c}", (128, SL), I16,
                                     kind="ExternalInput"))
    out_t = nc.dram_tensor("out", (ND, C), F32, kind="ExternalOutput")

    g_loc = [nc.dram_tensor(f"g{i}_loc", (16, SLAB), BF16) for i in (1, 2)]
    g_all = [nc.dram_tensor(f"g{i}_all", (NC, 16, SLAB), BF16,
                            addr_space="Shared") for i in (1, 2)]

    ctx = ExitStack()
    with ctx:
        tc = ctx.enter_context(tile.TileContext(nc))
        P1 = ctx.enter_context(tc.tile_pool(name="P1", bufs=1))
        P2 = ctx.enter_context(tc.tile_pool(name="P2", bufs=2))
        PSA = ctx.enter_context(tc.tile_pool(name="PSA", bufs=2, space="PSUM"))
        PSB = ctx.enter_context(tc.tile_pool(name="PSB", bufs=1, space="PSUM"))

        # ---- constants / weights (before ap_gather library load) ----
        ident = P1.tile([128, 128], F32)
        on1 = P2.tile([128, 128], F32, tag="on1", bufs=1)
        nc.vector.memset(on1[:], 1.0)
        nc.gpsimd.affine_select(ident[:], on1[:], [[1, 128]], AOT.is_equal,
                                0.0, base=0, channel_multiplier=-1)
        ones_sc = P1.tile([128, TILE_E], BF16)
        nc.vector.memset(ones_sc[:], 1.0)

        wW1 = P1.tile([128, cfg.KCH * H], BF16)
        wtmp = P2.tile([128, cfg.KCH * H], F32, tag="wtmp")
        for kb in range(cfg.KCH):
            nc.sync.dma_start(wtmp[:, kb * H:(kb + 1) * H],
                              w_t["W1"][kb * 128:(kb + 1) * 128, :])
        nc.vector.tensor_copy(wW1[:], wtmp[:])
        wsm = {}
        for nm, sh in [("W2", (H, C)), ("W3", (C, 32)), ("W4", (32, C))]:
            wf = P2.tile(list(sh), F32, tag="wtmp")
            nc.sync.dma_start(wf[:], w_t[nm][:])
            wsm[nm] = P1.tile(list(sh), BF16, name=f"wsm_{nm}", tag=f"wsm_{nm}")
            nc.vector.tensor_copy(wsm[nm][:], wf[:])
        vb = {}
        for nm, n in [("b1", H), ("b2", C), ("b3", 32), ("b4", C)]:
            vb[nm] = P1.tile([n, 1], F32, name=f"vb_{nm}", tag=f"vb_{nm}")
            nc.sync.dma_start(vb[nm][:], w_t[nm][:].unsqueeze(1))
        vb1r = P1.tile([128, 1], F32)
        vb2r = P1.tile([128, 1], F32)
        nc.vector.memset(vb1r[:], 0.0)
        nc.vector.memset(vb2r[:], 0.0)
        for q in range(QC):
            nc.sync.dma_start(vb1r[16 * q:16 * q + H, :], vb["b1"][:])
            nc.sync.dma_start(vb2r[16 * q:16 * q + C, :], vb["b2"][:])

        nc.gpsimd.load_library(library_config.ap_gather)

        posw_sb = []
        for c in range(NC):
            pt = P1.tile([128, SL // 16], I16, name=f"poswsb{c}", tag=f"poswsb{c}")
            nc.sync.dma_start(pt[:], posw_t[c][:])
            posw_sb.append(pt)
        NTMX16 = max(meta["ntot"]) // 16

        # ---- deg / dinv (slice layout) ----
        dinv = P1.tile([128, SL], F32)
        degf = P2.tile([128, SL], F32, tag="t0", bufs=1)
        nc.vector.memset(degf[:], 1.0)
        for c in range(NC):
            pp = P2.tile([128, SL], I16, tag="pp", bufs=1)
            nc.sync.dma_start(pp[:], posp_t[c][:])
            tmpp = P2.tile([128, SL], F32, tag="tmpp", bufs=1)
            nc.vector.tensor_copy(tmpp[:], pp[:])
            nc.vector.tensor_tensor(degf[:], degf[:], tmpp[:], AOT.add)
            nc.vector.tensor_tensor(degf[:, 1:SL], degf[:, 1:SL],
                                    tmpp[:, 0:SL - 1], AOT.subtract)
        sqd = P2.tile([128, SL], F32, tag="tmpp", bufs=1)
        nc.scalar.activation(sqd[:], degf[:], AFT.Sqrt)
        nc.vector.reciprocal(dinv[:], sqd[:])

        # ---- conv1 transform -> per-slice fm ministrip -> h1sl ----
        h1sl = P2.tile([128, SL], BF16, tag="hsl")
        NBLK = SL // 128
        with nc.named_scope("conv1mm"):
            for s in range(QC):
                # x arrives host-pre-transposed (feature-major, bf16): load
                # the 4 k-chunks straight into the matmul strip layout.
                xT = P1.tile([128, cfg.KCH * SL], BF16, tag="strip")
                for kb in range(cfg.KCH):
                    eng = (nc.sync, nc.scalar)[kb % 2]
                    eng.dma_start(
                        xT[:, kb * SL:(kb + 1) * SL],
                        x_t[kb * 128:(kb + 1) * 128, s * SL:(s + 1) * SL])
                h1q = P2.tile([16, SL], BF16, tag="fq")
                for n0 in range(0, SL, NWIN):
                    ph = PSA.tile([16, NWIN], F32, tag="ph")
                    for kb in range(cfg.KCH):
                        nc.tensor.matmul(
                            ph[:], wW1[:, kb * H:(kb + 1) * H],
                            xT[:, kb * SL + n0: kb * SL + n0 + NWIN],
                            start=(kb == 0), stop=(kb == cfg.KCH - 1))
                    nc.vector.tensor_copy(h1q[:, n0:n0 + NWIN], ph[:])
                # bridge transform-slice s (nodes [s*SL,(s+1)*SL)) into
                # balanced agg-slices (rank = node - q*SLQ)
                n0, n1 = s * SL, min((s + 1) * SL, SLAB)
                while n0 < n1:
                    q = min(n0 // SLQ, QC - 1)
                    qe = SLAB if q == QC - 1 else min((q + 1) * SLQ, SLAB)
                    pe = min(n1, qe)
                    nc.sync.dma_start(
                        h1sl[16 * q:16 * q + 16, n0 - q * SLQ: pe - q * SLQ],
                        h1q[:, n0 - s * SL: pe - s * SL])
                    n0 = pe

        # ---- aggregation ----
        tbl = P1.tile([128, TBLN], F32, name="tbl")
        nc.vector.memset(tbl[:, SLAB:], 0.0)
        TBH = (SLAB + 1) // 2
        tbs = P1.tile([128, TBH], BF16, name="tbs")
        HALVES = ((0, TBH), (TBH, SLAB))

        def dma_tbs(g_all_dram, c, h):
            # stage one bf16 half of chunk c's table (8 replica DMAs);
            # issued a chunk ahead so the expand-cast sits off the
            # gather critical path.
            lo, hi = HALVES[h]
            for r in range(NC):
                eng = (nc.sync, nc.scalar)[r % 2]
                eng.dma_start(out=tbs[16 * r:16 * r + 16, 0:hi - lo],
                              in_=g_all_dram[c, :, lo:hi])

        def dma_idx(c):
            it = P2.tile([128, NTMX16], I16, tag="idxs")
            nc.sync.dma_start(it[:, 0:meta["ntot"][c] // 16], idx_t[c][:])
            return it

        def cast_half(h):
            lo, hi = HALVES[h]
            nc.vector.tensor_copy(tbl[:, lo:hi], tbs[:, 0:hi - lo])

        def aggregate(g_all_dram, agg_out, scope):
            with nc.named_scope(scope):
                nc.vector.memset(agg_out[:], 0.0)
                idx_next = dma_idx(0)
                dma_tbs(g_all_dram, 0, 0)
                for c in range(NC):
                    ntot = meta["ntot"][c]
                    widths = meta["widths"][c]
                    idxs = idx_next
                    cast_half(0)
                    dma_tbs(g_all_dram, c, 1)
                    cast_half(1)
                    if c + 1 < NC:
                        dma_tbs(g_all_dram, c + 1, 0)
                        idx_next = dma_idx(c + 1)
                    strip = P1.tile([128, NTMX], F32, tag="strip")
                    off = 0
                    for ti, w in enumerate(widths):
                        gout = P2.tile([128, TILE_E], F32, tag="gout")
                        nc.gpsimd.ap_gather(
                            gout[:, 0:w], tbl[:], idxs[:, off // 16:(off + w) // 16],
                            channels=128, num_elems=TBLN, d=1, num_idxs=w)
                        init = 0.0 if ti == 0 else strip[:, off - 1:off]
                        nc.vector.tensor_tensor_scan(
                            strip[:, off:off + w], ones_sc[:, 0:w], gout[:, 0:w],
                            init, AOT.mult, AOT.add)
                        off += w
                    pw = posw_sb[c]
                    samp = P1.tile([128, SL], F32, tag="samp")
                    NPS = min(SL, pad_to(SLQ, 16))
                    if NPS < SL:
                        nc.vector.memset(samp[:, NPS:SL], 0.0)
                    nc.gpsimd.ap_gather(
                        samp[:, 0:NPS], strip[:, 0:ntot], pw[:, 0:NPS // 16],
                        channels=128, num_elems=ntot, d=1, num_idxs=NPS)
                    nc.vector.tensor_tensor(agg_out[:, 0:1], agg_out[:, 0:1],
                                            samp[:, 0:1], AOT.add)
                    dif = P2.tile([128, SL], F32, tag="tmpp", bufs=1)
                    nc.vector.tensor_tensor(dif[:, 1:SL], samp[:, 1:SL],
                                            samp[:, 0:SL - 1], AOT.subtract)
                    nc.vector.tensor_tensor(agg_out[:, 1:SL], agg_out[:, 1:SL],
                                            dif[:, 1:SL], AOT.add)

        def scale_and_ship(hsl, g_loc_d, g_all_d, agname):
            gsl = P2.tile([128, SL], BF16, tag="tmpp", bufs=1)
            nc.vector.tensor_tensor(gsl[:], hsl[:], dinv[:], AOT.mult)
            for q in range(QC):
                wv = ssize(q)
                nc.sync.dma_start(g_loc_d[:, q * SLQ: q * SLQ + wv],
                                  gsl[16 * q:16 * q + 16, 0:wv])
            with nc.named_scope(agname):
                nc.gpsimd.collective_compute(
                    "AllGather", AOT.bypass, replica_groups=[list(range(NC))],
                    ins=[g_loc_d[:]], outs=[g_all_d[:]])

        scale_and_ship(h1sl[:], g_loc[0], g_all[0], "ag1")
        agg = P1.tile([128, SL], F32, tag="agg")
        aggregate(g_all[0], agg[:], "agg1")

        # fin1: a1 = relu(dinv*(agg + dinv*h1) + b1)
        a1sl = P2.tile([128, SL], BF16, tag="hsl")
        t0 = P2.tile([128, SL], F32, tag="t0", bufs=1)
        nc.vector.tensor_tensor(t0[:], h1sl[:], dinv[:], AOT.mult)
        nc.vector.tensor_tensor(t0[:], t0[:], agg[:], AOT.add)
        nc.vector.tensor_tensor(t0[:], t0[:], dinv[:], AOT.mult)
        nc.vector.tensor_scalar(t0[:], t0[:], vb1r[:], None, AOT.add)
        nc.vector.tensor_scalar(a1sl[:], t0[:], 0.0, None, AOT.max)

        # ---- conv2 transform (per q mini-strip) -> h2sl ----
        h2sl = P2.tile([128, SL], BF16, tag="hsl")
        with nc.named_scope("conv2mm"):
            for q in range(QC):
                sq = ssize(q)
                a1q = P2.tile([16, SL], BF16, tag="fq")
                nc.sync.dma_start(a1q[:], a1sl[16 * q:16 * q + 16, :])
                h2q = P2.tile([16, SL], BF16, tag="fq")
                nc.vector.memset(h2q[:], 0.0)
                w0 = 0
                while w0 < sq:
                    wlen = min(NWIN, sq - w0)
                    ph2 = PSB.tile([C, NWIN], F32, tag="ph2")
                    nc.tensor.matmul(
                        ph2[0:C, 0:wlen], wsm["W2"][:],
                        a1q[:, w0:w0 + wlen], start=True, stop=True)
                    nc.vector.tensor_copy(h2q[0:C, w0:w0 + wlen], ph2[0:C, 0:wlen])
                    w0 += wlen
                nc.sync.dma_start(h2sl[16 * q:16 * q + 16, :], h2q[:])

        scale_and_ship(h2sl[:], g_loc[1], g_all[1], "ag2")
        aggregate(g_all[1], agg[:], "agg2")

        # fin2: o2 = dinv*(agg + dinv*h2) + b2  (no relu)
        o2sl = P2.tile([128, SL], F32, tag="t0", bufs=1)
        nc.vector.tensor_tensor(o2sl[:], h2sl[:], dinv[:], AOT.mult)
        nc.vector.tensor_tensor(o2sl[:], o2sl[:], agg[:], AOT.add)
        nc.vector.tensor_tensor(o2sl[:], o2sl[:], dinv[:], AOT.mult)
        nc.vector.tensor_scalar(o2sl[:], o2sl[:], vb2r[:], None, AOT.add)
        o2b = P2.tile([128, SL], BF16, tag="hsl")
        nc.vector.tensor_copy(o2b[:], o2sl[:])

        # ---- head + log_softmax per q ----
        with nc.named_scope("head"):
            for q in range(QC):
                sq = ssize(q)
                nblk = -(-sq // 128)
                o2q = P2.tile([16, SL], BF16, tag="fq")
                nc.sync.dma_start(o2q[:], o2b[16 * q:16 * q + 16, :])
                h4q = P2.tile([C, SL], F32, tag="t0", bufs=1)
                w0 = 0
                while w0 < sq:
                    wlen = min(NWIN, sq - w0)
                    ph3 = PSB.tile([32, NWIN], F32, tag="ph3")
                    nc.tensor.matmul(
                        ph3[0:32, 0:wlen], wsm["W3"][:],
                        o2q[0:C, w0:w0 + wlen], start=True, stop=True)
                    h3w = P2.tile([32, NWIN], BF16, tag="h3w", bufs=1)
                    t3 = P2.tile([32, NWIN], F32, tag="t3", bufs=1)
                    nc.vector.tensor_scalar(t3[:, 0:wlen], ph3[0:32, 0:wlen],
                                            vb["b3"][:], None, AOT.add)
                    nc.vector.tensor_scalar(h3w[:, 0:wlen], t3[:, 0:wlen],
                                            0.0, None, AOT.max)
                    ph4 = PSB.tile([C, NWIN], F32, tag="ph4")
                    nc.tensor.matmul(ph4[0:C, 0:wlen], wsm["W4"][:],
                                     h3w[:, 0:wlen], start=True, stop=True)
                    nc.vector.tensor_scalar(
                        h4q[:, w0:w0 + wlen], ph4[0:C, 0:wlen],
                        vb["b4"][:], None, AOT.add)
                    w0 += wlen
                lstr = P2.tile([128, nblk * C], F32, tag="lstr")
                nc.vector.memset(lstr[:], 0.0)
                for b in range(nblk):
                    blen = min(128, sq - b * 128)
                    ptr = PSB.tile([128, C], F32, tag="ptr")
                    nc.tensor.transpose(ptr[0:blen, :],
                                        h4q[:, b * 128: b * 128 + blen],
                                        ident[0:C, 0:C])
                    nc.vector.tensor_copy(lstr[0:blen, b * C:(b + 1) * C],
                                          ptr[0:blen, :])
                l3 = lstr[:].rearrange("p (b c) -> p b c", c=C)
                mx = P2.tile([128, nblk], F32, tag="mx")
                nc.vector.tensor_reduce(mx[:], l3, axis=mybir.AxisListType.X,
                                        op=AOT.max)
                zt = P2.tile([128, nblk * C], F32, tag="zt")
                z3 = zt[:].rearrange("p (b c) -> p b c", c=C)
                for ci in range(C):
                    nc.vector.tensor_tensor(z3[:, :, ci], l3[:, :, ci], mx[:],
                                            AOT.subtract)
                et = P2.tile([128, nblk * C], F32, tag="t3", bufs=1)
                nc.scalar.activation(et[:], zt[:], AFT.Exp)
                sz = P2.tile([128, nblk], F32, tag="mx")
                nc.vector.tensor_reduce(sz[:], et[:].rearrange("p (b c) -> p b c", c=C),
                                        axis=mybir.AxisListType.X, op=AOT.add)
                lsz = P2.tile([128, nblk], F32, tag="mx")
                nc.scalar.activation(lsz[:], sz[:], AFT.Ln)
                ot = P2.tile([128, nblk * C], F32, tag="zt")
                o3 = ot[:].rearrange("p (b c) -> p b c", c=C)
                for ci in range(C):
                    nc.vector.tensor_tensor(o3[:, :, ci], z3[:, :, ci], lsz[:],
                                            AOT.subtract)
                nfull = sq // 128
                if nfull:
                    nc.sync.dma_start(
                        out_t[q * SLQ: q * SLQ + nfull * 128, :].rearrange(
                            "(b p) c -> p b c", p=128),
                        ot[:, 0:nfull * C].rearrange("p (b c) -> p b c", c=C))
                tail = sq - nfull * 128
                if tail:
                    nc.sync.dma_start(
                        out_t[q * SLQ + nfull * 128: q * SLQ + sq, :],
                        ot[0:tail, nfull * C:(nfull + 1) * C])

    nc.compile()
    return nc


# ---------------------------------------------------------------------------
# host-side sharding + compile cache + runner
# ---------------------------------------------------------------------------
N_NODES, F_IN_C, H_C, C_C = 100000, 512, 16, 5
_cache = {}

last_exec_time_ns = None


def kernel(x, edge_index, W1, b1, W2, b2, W3, b3, W4, b4):
    global last_exec_time_ns
    from concourse import bass_utils

    x = np.ascontiguousarray(np.asarray(x, np.float32))
    edge_index = np.asarray(edge_index)
    assert x.shape == (N_NODES, F_IN_C), x.shape
    cfg = Cfg(N_NODES, F_IN_C, H_C, C_C)
    meta, per_core = host_prep(edge_index, cfg)
    key = tuple(meta["ntot"])
    if key not in _cache:
        _cache[key] = build(cfg, meta)
    nc = _cache[key]

    wts = dict(W1=np.asarray(W1, np.float32), b1=np.asarray(b1, np.float32),
               W2=np.asarray(W2, np.float32), b2=np.asarray(b2, np.float32),
               W3=np.asarray(W3, np.float32), b3=np.asarray(b3, np.float32),
               W4=np.asarray(W4, np.float32), b4=np.asarray(b4, np.float32))
    import ml_dtypes
    in_maps = []
    for k in range(NC):
        lo = k * cfg.SLAB
        hi = min((k + 1) * cfg.SLAB, N_NODES)
        xs = np.zeros((F_IN_C, cfg.ND), ml_dtypes.bfloat16)
        xs[:, :hi - lo] = x[lo:hi].astype(ml_dtypes.bfloat16).T
        im = dict(x=xs, **wts)
        im.update(per_core[k])
        in_maps.append(im)

    res = bass_utils.run_bass_kernel_spmd(
        nc, in_maps, list(range(NC)), trace=_HOOK_OK,
        trace_cores=[0] if _HOOK_OK else None)
    last_exec_time_ns = res.exec_time_ns

    outs = []
    for k in range(NC):
        lo = k * cfg.SLAB
        hi = min((k + 1) * cfg.SLAB, N_NODES)
        outs.append(res.results[k]["out"][:hi - lo])
    return np.ascontiguousarray(np.concatenate(outs).astype(np.float32))



# revision 20
# speedup vs baseline: 1.0098x; 1.0098x over previous
"""Self-contained Trainium2 Bass kernel for the 2-layer GCN problem.

kernel(**inputs) -> np.ndarray [100000, 5] float32 (log-softmax outputs).

Strategy: destination-node slabs of 12500 across the 8 NeuronCores.
x arrives host-pre-transposed to feature-major bf16 (F, ND) so conv1
loads straight into the PE strip layout; activations ship through the
AllGather as bf16 and are expanded to the fp32 gather table on the DVE
via a half-slab staging tile.  Graph aggregation per source-slab chunk:
GPSIMD ap_gather from the table (replicated over the 8 Q7 core groups),
chained fp32 running cumsum on the DVE, one boundary ap_gather at
per-node last-edge positions, adjacent diff.  Degrees are derived
on-device from the dst-segment position tensors.  The host does index
preprocessing (sorting/partitioning edge_index per the sharding
contract), the x transpose/cast, and input sharding/unsharding.
"""
import os, sys
sys.path.insert(0, "/opt/trn_rl_repo")

_HOOK_OK = False


def _install_ntff_hook():
    global _HOOK_OK
    try:
        import types
        import antenv
        mod = types.ModuleType("antenv.axon_hooks")
        _h = {}
        mod.set_axon_ntff_profile_hook = lambda h: _h.__setitem__("h", h)
        mod.get_axon_ntff_profile_hook = lambda: _h.get("h")
        sys.modules["antenv.axon_hooks"] = mod
        antenv.axon_hooks = mod
        from trn_agent_boot.trn_boot import _ntff_profile_via_ctypes
        mod.set_axon_ntff_profile_hook(
            _ntff_profile_via_ctypes("/opt/axon/libaxon_pjrt.so"))
        from concourse import bass_utils as _bu
        _bu.upload_artifacts = lambda tmpdir: tmpdir
        _HOOK_OK = True
    except Exception:
        _HOOK_OK = False


_install_ntff_hook()

"""builder"""
import numpy as np
from contextlib import ExitStack

import concourse.bacc as bacc
import concourse.bass as bass
import concourse.mybir as mybir
import concourse.tile as tile
from concourse import library_config

F32 = mybir.dt.float32
BF16 = mybir.dt.bfloat16
I16 = mybir.dt.int16
AOT = mybir.AluOpType
AFT = mybir.ActivationFunctionType

NC = 8
QC = 8
TILE_E = 2560


def pad_to(x, m):
    return (x + m - 1) // m * m


class Cfg:
    def __init__(self, n_nodes, f_in=512, h=16, c=5):
        self.N = n_nodes
        self.SLAB = -(-n_nodes // NC)
        self.SL = pad_to(-(-self.SLAB // QC), 128)
        self.ND = self.SL * QC
        self.SLQ = -(-self.SLAB // QC)         # balanced agg-slice size
        self.F_IN, self.H, self.C = f_in, h, c
        self.KCH = f_in // 128
        nwin = None
        for cand in (512, 448, 416, 384, 320, 256, 128):
            if self.SL % cand == 0:
                nwin = cand
                break
        assert nwin, self.SL
        self.NWIN = nwin


def host_prep(edge_index, cfg):
    src = np.asarray(edge_index[0], np.int64)
    dst = np.asarray(edge_index[1], np.int64)
    cores = []
    for k in range(NC):
        d_lo = k * cfg.SLAB
        d_hi = min((k + 1) * cfg.SLAB, cfg.N)
        sel = (dst >= d_lo) & (dst < d_hi)
        s, d = src[sel], dst[sel] - d_lo
        chunks = []
        for c in range(NC):
            lo = c * cfg.SLAB
            hi = min(lo + cfg.SLAB, cfg.N)
            m = (s >= lo) & (s < hi)
            sc, dc = s[m] - lo, d[m]
            o = np.argsort(dc, kind="stable")
            chunks.append((sc[o].astype(np.int64), dc[o].astype(np.int64)))
        cores.append(chunks)
    meta = dict(ntot=[], widths=[])
    for c in range(NC):
        maxe = 1
        for k in range(NC):
            sc, dc = cores[k][c]
            if len(dc):
                maxe = max(maxe, int(np.bincount(
                    np.minimum(dc // cfg.SLQ, QC - 1), minlength=QC).max()))
        ntot = pad_to(maxe + 1, 16)
        # split into ~equal %16 tiles of at most TILE_E
        nt = -(-ntot // TILE_E)
        widths = []
        rem = ntot
        for t in range(nt, 0, -1):
            w = pad_to(-(-rem // t), 16) if t > 1 else rem
            w = min(w, rem)
            widths.append(w)
            rem -= w
        assert sum(widths) == ntot and all(x % 16 == 0 for x in widths)
        meta["ntot"].append(ntot)
        meta["widths"].append(widths)
    per_core = []
    for k in range(NC):
        ins = {}
        for c in range(NC):
            ntot = meta["ntot"][c]
            sc, dc = cores[k][c]
            idxw = np.zeros((128, ntot // 16), np.int16)
            posw = np.zeros((128, cfg.SL // 16), np.int16)
            posp = np.zeros((128, cfg.SL), np.int16)
            for q in range(QC):
                a = q * cfg.SLQ
                b = min((q + 1) * cfg.SLQ, cfg.SLAB) if q < QC - 1 else cfg.SLAB
                mm = (dc >= a) & (dc < b)
                sq = sc[mm]
                dq = dc[mm] - a
                idx = np.full(ntot, cfg.SLAB, np.int64)
                idx[1:1 + len(sq)] = sq
                pos = np.cumsum(np.bincount(dq, minlength=cfg.SL))
                idxw[16 * q:16 * (q + 1)] = idx.reshape(-1, 16).T.astype(np.int16)
                posw[16 * q:16 * (q + 1)] = pos.reshape(-1, 16).T.astype(np.int16)
                posp[16 * q:16 * (q + 1)] = pos[None, :].astype(np.int16)
            ins[f"idx{c}"] = idxw
            ins[f"posw{c}"] = posw
            ins[f"posp{c}"] = posp
        per_core.append(ins)
    return meta, per_core


def build(cfg, meta):
    nc = bacc.Bacc("TRN2", target_bir_lowering=False)
    F, H, C = cfg.F_IN, cfg.H, cfg.C
    ND, SL, SLAB, NWIN = cfg.ND, cfg.SL, cfg.SLAB, cfg.NWIN
    SLQ = cfg.SLQ
    def ssize(q):
        return (SLAB - q * SLQ) if q == QC - 1 else min(SLQ, SLAB - q * SLQ)
    TBLN = SLAB + 16
    NTMX = max(meta["ntot"])

    x_t = nc.dram_tensor("x", (F, ND), BF16, kind="ExternalInput")
    w_t = {}
    for nm, sh in [("W1", (F, H)), ("b1", (H,)), ("W2", (H, C)), ("b2", (C,)),
                   ("W3", (C, 32)), ("b3", (32,)), ("W4", (32, C)), ("b4", (C,))]:
        w_t[nm] = nc.dram_tensor(nm, sh, F32, kind="ExternalInput")
    idx_t, posw_t, posp_t = [], [], []
    for c in range(NC):
        idx_t.append(nc.dram_tensor(f"idx{c}", (128, meta["ntot"][c] // 16), I16,
                                    kind="ExternalInput"))
        posw_t.append(nc.dram_tensor(f"posw{c}", (128, SL // 16), I16,
                                     kind="ExternalInput"))
        posp_t.append(nc.dram_tensor(f"posp{c}", (128, SL), I16,
                                     kind="ExternalInput"))
    out_t = nc.dram_tensor("out", (ND, C), F32, kind="ExternalOutput")

    g_loc = [nc.dram_tensor(f"g{i}_loc", (16, SLAB), BF16) for i in (1, 2)]
    g_all = [nc.dram_tensor(f"g{i}_all", (NC, 16, SLAB), BF16,
                            addr_space="Shared") for i in (1, 2)]

    ctx = ExitStack()
    with ctx:
        tc = ctx.enter_context(tile.TileContext(nc))
        P1 = ctx.enter_context(tc.tile_pool(name="P1", bufs=1))
        P2 = ctx.enter_context(tc.tile_pool(name="P2", bufs=2))
        PSA = ctx.enter_context(tc.tile_pool(name="PSA", bufs=2, space="PSUM"))
        PSB = ctx.enter_context(tc.tile_pool(name="PSB", bufs=1, space="PSUM"))

        # ---- constants / weights (before ap_gather library load) ----
        ident = P1.tile([128, 128], F32)
        on1 = P2.tile([128, 128], F32, tag="on1", bufs=1)
        nc.vector.memset(on1[:], 1.0)
        nc.gpsimd.affine_select(ident[:], on1[:], [[1, 128]], AOT.is_equal,
                                0.0, base=0, channel_multiplier=-1)
        ones_sc = P1.tile([128, TILE_E], BF16)
        nc.vector.memset(ones_sc[:], 1.0)

        wW1 = P1.tile([128, cfg.KCH * H], BF16)
        wtmp = P2.tile([128, cfg.KCH * H], F32, tag="wtmp")
        for kb in range(cfg.KCH):
            nc.sync.dma_start(wtmp[:, kb * H:(kb + 1) * H],
                              w_t["W1"][kb * 128:(kb + 1) * 128, :])
        nc.vector.tensor_copy(wW1[:], wtmp[:])
        wsm = {}
        for nm, sh in [("W2", (H, C)), ("W3", (C, 32)), ("W4", (32, C))]:
            wf = P2.tile(list(sh), F32, tag="wtmp")
            nc.sync.dma_start(wf[:], w_t[nm][:])
            wsm[nm] = P1.tile(list(sh), BF16, name=f"wsm_{nm}", tag=f"wsm_{nm}")
            nc.vector.tensor_copy(wsm[nm][:], wf[:])
        vb = {}
        for nm, n in [("b1", H), ("b2", C), ("b3", 32), ("b4", C)]:
            vb[nm] = P1.tile([n, 1], F32, name=f"vb_{nm}", tag=f"vb_{nm}")
            nc.sync.dma_start(vb[nm][:], w_t[nm][:].unsqueeze(1))
        vb1r = P1.tile([128, 1], F32)
        vb2r = P1.tile([128, 1], F32)
        nc.vector.memset(vb1r[:], 0.0)
        nc.vector.memset(vb2r[:], 0.0)
        for q in range(QC):
            nc.sync.dma_start(vb1r[16 * q:16 * q + H, :], vb["b1"][:])
            nc.sync.dma_start(vb2r[16 * q:16 * q + C, :], vb["b2"][:])

        nc.gpsimd.load_library(library_config.ap_gather)

        idx_sb, posw_sb = [], []
        for c in range(NC):
            it = P1.tile([128, meta["ntot"][c] // 16], I16, name=f"idxsb{c}",
                         tag=f"idxsb{c}")
            nc.sync.dma_start(it[:], idx_t[c][:])
            idx_sb.append(it)
            pt = P1.tile([128, SL // 16], I16, name=f"poswsb{c}", tag=f"poswsb{c}")
            nc.sync.dma_start(pt[:], posw_t[c][:])
            posw_sb.append(pt)
        NTMX16 = max(meta["ntot"]) // 16

        # ---- deg / dinv (slice layout) ----
        dinv = P1.tile([128, SL], F32)
        degf = P2.tile([128, SL], F32, tag="t0", bufs=1)
        nc.vector.memset(degf[:], 1.0)
        for c in range(NC):
            pp = P2.tile([128, SL], I16, tag="pp", bufs=1)
            nc.sync.dma_start(pp[:], posp_t[c][:])
            tmpp = P2.tile([128, SL], F32, tag="tmpp", bufs=1)
            nc.vector.tensor_copy(tmpp[:], pp[:])
            nc.vector.tensor_tensor(degf[:], degf[:], tmpp[:], AOT.add)
            nc.vector.tensor_tensor(degf[:, 1:SL], degf[:, 1:SL],
                                    tmpp[:, 0:SL - 1], AOT.subtract)
        sqd = P2.tile([128, SL], F32, tag="tmpp", bufs=1)
        nc.scalar.activation(sqd[:], degf[:], AFT.Sqrt)
        nc.vector.reciprocal(dinv[:], sqd[:])

        # ---- conv1 transform -> per-slice fm ministrip -> h1sl ----
        h1sl = P2.tile([128, SL], BF16, tag="hsl")
        NBLK = SL // 128
        with nc.named_scope("conv1mm"):
            for s in range(QC):
                # x arrives host-pre-transposed (feature-major, bf16): load
                # the 4 k-chunks straight into the matmul strip layout.
                xT = P1.tile([128, cfg.KCH * SL], BF16, tag="strip")
                for kb in range(cfg.KCH):
                    eng = (nc.sync, nc.scalar)[kb % 2]
                    eng.dma_start(
                        xT[:, kb * SL:(kb + 1) * SL],
                        x_t[kb * 128:(kb + 1) * 128, s * SL:(s + 1) * SL])
                h1q = P2.tile([16, SL], BF16, tag="fq")
                for n0 in range(0, SL, NWIN):
                    ph = PSA.tile([16, NWIN], F32, tag="ph")
                    for kb in range(cfg.KCH):
                        nc.tensor.matmul(
                            ph[:], wW1[:, kb * H:(kb + 1) * H],
                            xT[:, kb * SL + n0: kb * SL + n0 + NWIN],
                            start=(kb == 0), stop=(kb == cfg.KCH - 1))
                    nc.vector.tensor_copy(h1q[:, n0:n0 + NWIN], ph[:])
                # bridge transform-slice s (nodes [s*SL,(s+1)*SL)) into
                # balanced agg-slices (rank = node - q*SLQ)
                n0, n1 = s * SL, min((s + 1) * SL, SLAB)
                while n0 < n1:
                    q = min(n0 // SLQ, QC - 1)
                    qe = SLAB if q == QC - 1 else min((q + 1) * SLQ, SLAB)
                    pe = min(n1, qe)
                    nc.sync.dma_start(
                        h1sl[16 * q:16 * q + 16, n0 - q * SLQ: pe - q * SLQ],
                        h1q[:, n0 - s * SL: pe - s * SL])
                    n0 = pe

        # ---- aggregation ----
        tbl = P1.tile([128, TBLN], F32, name="tbl")
        nc.vector.memset(tbl[:, SLAB:], 0.0)
        TBH = (SLAB + 1) // 2
        tbs = P1.tile([128, TBH], BF16, name="tbs")
        HALVES = ((0, TBH), (TBH, SLAB))

        def dma_tbs(g_all_dram, c, h):
            # stage one bf16 half of chunk c's table (8 replica DMAs);
            # issued a chunk ahead so the expand-cast sits off the
            # gather critical path.
            lo, hi = HALVES[h]
            for r in range(NC):
                eng = (nc.sync, nc.scalar)[r % 2]
                eng.dma_start(out=tbs[16 * r:16 * r + 16, 0:hi - lo],
                              in_=g_all_dram[c, :, lo:hi])

        def dma_idx(c):
            it = P2.tile([128, NTMX16], I16, tag="idxs")
            nc.sync.dma_start(it[:, 0:meta["ntot"][c] // 16], idx_t[c][:])
            return it

        def cast_half(h):
            lo, hi = HALVES[h]
            nc.vector.tensor_copy(tbl[:, lo:hi], tbs[:, 0:hi - lo])

        def aggregate(g_all_dram, agg_out, scope):
            with nc.named_scope(scope):
                nc.vector.memset(agg_out[:], 0.0)
                for c in range(NC):
                    ntot = meta["ntot"][c]
                    widths = meta["widths"][c]
                    idxs = idx_sb[c]
                    dma_tbs(g_all_dram, c, 0)
                    cast_half(0)
                    dma_tbs(g_all_dram, c, 1)
                    cast_half(1)
                    strip = P1.tile([128, NTMX], F32, tag="strip")
                    off = 0
                    for ti, w in enumerate(widths):
                        gout = P2.tile([128, TILE_E], F32, tag="gout")
                        nc.gpsimd.ap_gather(
                            gout[:, 0:w], tbl[:], idxs[:, off // 16:(off + w) // 16],
                            channels=128, num_elems=TBLN, d=1, num_idxs=w)
                        init = 0.0 if ti == 0 else strip[:, off - 1:off]
                        nc.vector.tensor_tensor_scan(
                            strip[:, off:off + w], ones_sc[:, 0:w], gout[:, 0:w],
                            init, AOT.mult, AOT.add)
                        off += w
                    pw = posw_sb[c]
                    samp = P1.tile([128, SL], F32, tag="samp")
                    NPS = min(SL, pad_to(SLQ, 16))
                    if NPS < SL:
                        nc.vector.memset(samp[:, NPS:SL], 0.0)
                    nc.gpsimd.ap_gather(
                        samp[:, 0:NPS], strip[:, 0:ntot], pw[:, 0:NPS // 16],
                        channels=128, num_elems=ntot, d=1, num_idxs=NPS)
                    nc.vector.tensor_tensor(agg_out[:, 0:1], agg_out[:, 0:1],
                                            samp[:, 0:1], AOT.add)
                    dif = P2.tile([128, SL], F32, tag="tmpp", bufs=1)
                    nc.vector.tensor_tensor(dif[:, 1:SL], samp[:, 1:SL],
                                            samp[:, 0:SL - 1], AOT.subtract)
                    nc.vector.tensor_tensor(agg_out[:, 1:SL], agg_out[:, 1:SL],
                                            dif[:, 1:SL], AOT.add)

        def scale_and_ship(hsl, g_loc_d, g_all_d, agname):
            gsl = P2.tile([128, SL], BF16, tag="tmpp", bufs=1)
            nc.vector.tensor_tensor(gsl[:], hsl[:], dinv[:], AOT.mult)
            for q in range(QC):
                wv = ssize(q)
                nc.sync.dma_start(g_loc_d[:, q * SLQ: q * SLQ + wv],
                                  gsl[16 * q:16 * q + 16, 0:wv])
            with nc.named_scope(agname):
                nc.gpsimd.collective_compute(
                    "AllGather", AOT.bypass, replica_groups=[list(range(NC))],
                    ins=[g_loc_d[:]], outs=[g_all_d[:]])

        scale_and_ship(h1sl[:], g_loc[0], g_all[0], "ag1")
        agg = P1.tile([128, SL], F32, tag="agg")
        aggregate(g_all[0], agg[:], "agg1")

        # fin1: a1 = relu(dinv*(agg + dinv*h1) + b1)
        a1sl = P2.tile([128, SL], BF16, tag="hsl")
        t0 = P2.tile([128, SL], F32, tag="t0", bufs=1)
        nc.vector.tensor_tensor(t0[:], h1sl[:], dinv[:], AOT.mult)
        nc.vector.tensor_tensor(t0[:], t0[:], agg[:], AOT.add)
        nc.vector.tensor_tensor(t0[:], t0[:], dinv[:], AOT.mult)
        nc.vector.tensor_scalar(t0[:], t0[:], vb1r[:], None, AOT.add)
        nc.vector.tensor_scalar(a1sl[:], t0[:], 0.0, None, AOT.max)

        # ---- conv2 transform (per q mini-strip) -> h2sl ----
        h2sl = P2.tile([128, SL], BF16, tag="hsl")
        with nc.named_scope("conv2mm"):
            for q in range(QC):
                sq = ssize(q)
                a1q = P2.tile([16, SL], BF16, tag="fq")
                nc.sync.dma_start(a1q[:], a1sl[16 * q:16 * q + 16, :])
                h2q = P2.tile([16, SL], BF16, tag="fq")
                nc.vector.memset(h2q[:], 0.0)
                w0 = 0
                while w0 < sq:
                    wlen = min(NWIN, sq - w0)
                    ph2 = PSB.tile([C, NWIN], F32, tag="ph2")
                    nc.tensor.matmul(
                        ph2[0:C, 0:wlen], wsm["W2"][:],
                        a1q[:, w0:w0 + wlen], start=True, stop=True)
                    nc.vector.tensor_copy(h2q[0:C, w0:w0 + wlen], ph2[0:C, 0:wlen])
                    w0 += wlen
                nc.sync.dma_start(h2sl[16 * q:16 * q + 16, :], h2q[:])

        scale_and_ship(h2sl[:], g_loc[1], g_all[1], "ag2")
        aggregate(g_all[1], agg[:], "agg2")

        # fin2: o2 = dinv*(agg + dinv*h2) + b2  (no relu)
        o2sl = P2.tile([128, SL], F32, tag="t0", bufs=1)
        nc.vector.tensor_tensor(o2sl[:], h2sl[:], dinv[:], AOT.mult)
        nc.vector.tensor_tensor(o2sl[:], o2sl[:], agg[:], AOT.add)
        nc.vector.tensor_tensor(o2sl[:], o2sl[:], dinv[:], AOT.mult)
        nc.vector.tensor_scalar(o2sl[:], o2sl[:], vb2r[:], None, AOT.add)
        o2b = P2.tile([128, SL], BF16, tag="hsl")
        nc.vector.tensor_copy(o2b[:], o2sl[:])

        # ---- head + log_softmax per q ----
        with nc.named_scope("head"):
            for q in range(QC):
                sq = ssize(q)
                nblk = -(-sq // 128)
                o2q = P2.tile([16, SL], BF16, tag="fq")
                nc.sync.dma_start(o2q[:], o2b[16 * q:16 * q + 16, :])
                h4q = P2.tile([C, SL], F32, tag="t0", bufs=1)
                w0 = 0
                while w0 < sq:
                    wlen = min(NWIN, sq - w0)
                    ph3 = PSB.tile([32, NWIN], F32, tag="ph3")
                    nc.tensor.matmul(
                        ph3[0:32, 0:wlen], wsm["W3"][:],
                        o2q[0:C, w0:w0 + wlen], start=True, stop=True)
                    h3w = P2.tile([32, NWIN], BF16, tag="h3w", bufs=1)
                    t3 = P2.tile([32, NWIN], F32, tag="t3", bufs=1)
                    nc.vector.tensor_scalar(t3[:, 0:wlen], ph3[0:32, 0:wlen],
                                            vb["b3"][:], None, AOT.add)
                    nc.vector.tensor_scalar(h3w[:, 0:wlen], t3[:, 0:wlen],
                                            0.0, None, AOT.max)
                    ph4 = PSB.tile([C, NWIN], F32, tag="ph4")
                    nc.tensor.matmul(ph4[0:C, 0:wlen], wsm["W4"][:],
                                     h3w[:, 0:wlen], start=True, stop=True)
                    nc.vector.tensor_scalar(
                        h4q[:, w0:w0 + wlen], ph4[0:C, 0:wlen],
                        vb["b4"][:], None, AOT.add)
                    w0 += wlen
                lstr = P2.tile([128, nblk * C], F32, tag="lstr")
                nc.vector.memset(lstr[:], 0.0)
                for b in range(nblk):
                    blen = min(128, sq - b * 128)
                    ptr = PSB.tile([128, C], F32, tag="ptr")
                    nc.tensor.transpose(ptr[0:blen, :],
                                        h4q[:, b * 128: b * 128 + blen],
                                        ident[0:C, 0:C])
                    nc.vector.tensor_copy(lstr[0:blen, b * C:(b + 1) * C],
                                          ptr[0:blen, :])
                l3 = lstr[:].rearrange("p (b c) -> p b c", c=C)
                mx = P2.tile([128, nblk], F32, tag="mx")
                nc.vector.tensor_reduce(mx[:], l3, axis=mybir.AxisListType.X,
                                        op=AOT.max)
                zt = P2.tile([128, nblk * C], F32, tag="zt")
                z3 = zt[:].rearrange("p (b c) -> p b c", c=C)
                for ci in range(C):
                    nc.vector.tensor_tensor(z3[:, :, ci], l3[:, :, ci], mx[:],
                                            AOT.subtract)
                et = P2.tile([128, nblk * C], F32, tag="t3", bufs=1)
                nc.scalar.activation(et[:], zt[:], AFT.Exp)
                sz = P2.tile([128, nblk], F32, tag="mx")
                nc.vector.tensor_reduce(sz[:], et[:].rearrange("p (b c) -> p b c", c=C),
                                        axis=mybir.AxisListType.X, op=AOT.add)
                lsz = P2.tile([128, nblk], F32, tag="mx")
                nc.scalar.activation(lsz[:], sz[:], AFT.Ln)
                ot = P2.tile([128, nblk * C], F32, tag="zt")
                o3 = ot[:].rearrange("p (b c) -> p b c", c=C)
                for ci in range(C):
                    nc.vector.tensor_tensor(o3[:, :, ci], z3[:, :, ci], lsz[:],
                                            AOT.subtract)
                nfull = sq // 128
                if nfull:
                    nc.sync.dma_start(
                        out_t[q * SLQ: q * SLQ + nfull * 128, :].rearrange(
                            "(b p) c -> p b c", p=128),
                        ot[:, 0:nfull * C].rearrange("p (b c) -> p b c", c=C))
                tail = sq - nfull * 128
                if tail:
                    nc.sync.dma_start(
                        out_t[q * SLQ + nfull * 128: q * SLQ + sq, :],
                        ot[0:tail, nfull * C:(nfull + 1) * C])

    nc.compile()
    return nc


# ---------------------------------------------------------------------------
# host-side sharding + compile cache + runner
# ---------------------------------------------------------------------------
N_NODES, F_IN_C, H_C, C_C = 100000, 512, 16, 5
_cache = {}

last_exec_time_ns = None


def kernel(x, edge_index, W1, b1, W2, b2, W3, b3, W4, b4):
    global last_exec_time_ns
    from concourse import bass_utils

    x = np.ascontiguousarray(np.asarray(x, np.float32))
    edge_index = np.asarray(edge_index)
    assert x.shape == (N_NODES, F_IN_C), x.shape
    cfg = Cfg(N_NODES, F_IN_C, H_C, C_C)
    meta, per_core = host_prep(edge_index, cfg)
    key = tuple(meta["ntot"])
    if key not in _cache:
        _cache[key] = build(cfg, meta)
    nc = _cache[key]

    wts = dict(W1=np.asarray(W1, np.float32), b1=np.asarray(b1, np.float32),
               W2=np.asarray(W2, np.float32), b2=np.asarray(b2, np.float32),
               W3=np.asarray(W3, np.float32), b3=np.asarray(b3, np.float32),
               W4=np.asarray(W4, np.float32), b4=np.asarray(b4, np.float32))
    import ml_dtypes
    in_maps = []
    for k in range(NC):
        lo = k * cfg.SLAB
        hi = min((k + 1) * cfg.SLAB, N_NODES)
        xs = np.zeros((F_IN_C, cfg.ND), ml_dtypes.bfloat16)
        xs[:, :hi - lo] = x[lo:hi].astype(ml_dtypes.bfloat16).T
        im = dict(x=xs, **wts)
        im.update(per_core[k])
        in_maps.append(im)

    res = bass_utils.run_bass_kernel_spmd(
        nc, in_maps, list(range(NC)), trace=_HOOK_OK,
        trace_cores=[0] if _HOOK_OK else None)
    last_exec_time_ns = res.exec_time_ns

    outs = []
    for k in range(NC):
        lo = k * cfg.SLAB
        hi = min((k + 1) * cfg.SLAB, N_NODES)
        outs.append(res.results[k]["out"][:hi - lo])
    return np.ascontiguousarray(np.concatenate(outs).astype(np.float32))



# revision 23
# speedup vs baseline: 1.0146x; 1.0047x over previous
"""Self-contained Trainium2 Bass kernel for the 2-layer GCN problem.

kernel(**inputs) -> np.ndarray [100000, 5] float32 (log-softmax outputs).

Strategy: destination-node slabs of 12500 across the 8 NeuronCores.
x arrives host-pre-transposed to feature-major bf16 (F, ND) so conv1
loads straight into the PE strip layout; activations ship through the
AllGather as bf16 and are expanded to the fp32 gather table on the DVE
via a half-slab staging tile.  Graph aggregation per source-slab chunk:
GPSIMD ap_gather from the table (replicated over the 8 Q7 core groups),
chained fp32 running cumsum on the DVE, one boundary ap_gather at
per-node last-edge positions, adjacent diff.  Degrees are derived
on-device from the dst-segment position tensors.  The host does index
preprocessing (sorting/partitioning edge_index per the sharding
contract), the x transpose/cast, and input sharding/unsharding.
"""
import os, sys
sys.path.insert(0, "/opt/trn_rl_repo")

_HOOK_OK = False


def _install_ntff_hook():
    global _HOOK_OK
    try:
        import types
        import antenv
        mod = types.ModuleType("antenv.axon_hooks")
        _h = {}
        mod.set_axon_ntff_profile_hook = lambda h: _h.__setitem__("h", h)
        mod.get_axon_ntff_profile_hook = lambda: _h.get("h")
        sys.modules["antenv.axon_hooks"] = mod
        antenv.axon_hooks = mod
        from trn_agent_boot.trn_boot import _ntff_profile_via_ctypes
        mod.set_axon_ntff_profile_hook(
            _ntff_profile_via_ctypes("/opt/axon/libaxon_pjrt.so"))
        from concourse import bass_utils as _bu
        _bu.upload_artifacts = lambda tmpdir: tmpdir
        _HOOK_OK = True
    except Exception:
        _HOOK_OK = False


_install_ntff_hook()

"""builder"""
import numpy as np
from contextlib import ExitStack

import concourse.bacc as bacc
import concourse.bass as bass
import concourse.mybir as mybir
import concourse.tile as tile
from concourse import library_config

F32 = mybir.dt.float32
BF16 = mybir.dt.bfloat16
I16 = mybir.dt.int16
AOT = mybir.AluOpType
AFT = mybir.ActivationFunctionType

NC = 8
QC = 8
TILE_E = 2560


def pad_to(x, m):
    return (x + m - 1) // m * m


class Cfg:
    def __init__(self, n_nodes, f_in=512, h=16, c=5):
        self.N = n_nodes
        self.SLAB = -(-n_nodes // NC)
        self.SL = pad_to(-(-self.SLAB // QC), 128)
        self.ND = self.SL * QC
        self.SLQ = -(-self.SLAB // QC)         # balanced agg-slice size
        self.F_IN, self.H, self.C = f_in, h, c
        self.KCH = f_in // 128
        nwin = None
        for cand in (512, 448, 416, 384, 320, 256, 128):
            if self.SL % cand == 0:
                nwin = cand
                break
        assert nwin, self.SL
        self.NWIN = nwin


def host_prep(edge_index, cfg):
    src = np.asarray(edge_index[0], np.int64)
    dst = np.asarray(edge_index[1], np.int64)
    cores = []
    for k in range(NC):
        d_lo = k * cfg.SLAB
        d_hi = min((k + 1) * cfg.SLAB, cfg.N)
        sel = (dst >= d_lo) & (dst < d_hi)
        s, d = src[sel], dst[sel] - d_lo
        chunks = []
        for c in range(NC):
            lo = c * cfg.SLAB
            hi = min(lo + cfg.SLAB, cfg.N)
            m = (s >= lo) & (s < hi)
            sc, dc = s[m] - lo, d[m]
            o = np.argsort(dc, kind="stable")
            chunks.append((sc[o].astype(np.int64), dc[o].astype(np.int64)))
        cores.append(chunks)
    meta = dict(ntot=[], widths=[])
    for c in range(NC):
        maxe = 1
        for k in range(NC):
            sc, dc = cores[k][c]
            if len(dc):
                maxe = max(maxe, int(np.bincount(
                    np.minimum(dc // cfg.SLQ, QC - 1), minlength=QC).max()))
        ntot = pad_to(maxe + 1, 16)
        # split into ~equal tiles of at most TILE_E; non-final widths must be
        # %64 so every tile's idx-slice starts 4B-aligned (the Q7 idx
        # preload reads the int16 stream as 32-bit words).
        nt = -(-ntot // TILE_E)
        widths = []
        rem = ntot
        for t in range(nt, 0, -1):
            w = min(pad_to(-(-rem // t), 64), TILE_E) if t > 1 else rem
            w = min(w, rem)
            widths.append(w)
            rem -= w
        assert sum(widths) == ntot and all(x % 16 == 0 for x in widths)
        assert all(x % 64 == 0 for x in widths[:-1])
        meta["ntot"].append(ntot)
        meta["widths"].append(widths)
    per_core = []
    for k in range(NC):
        ins = {}
        for c in range(NC):
            ntot = meta["ntot"][c]
            sc, dc = cores[k][c]
            idxw = np.zeros((128, ntot // 16), np.int16)
            posw = np.zeros((128, cfg.SL // 16), np.int16)
            posp = np.zeros((128, cfg.SL), np.int16)
            for q in range(QC):
                a = q * cfg.SLQ
                b = min((q + 1) * cfg.SLQ, cfg.SLAB) if q < QC - 1 else cfg.SLAB
                mm = (dc >= a) & (dc < b)
                sq = sc[mm]
                dq = dc[mm] - a
                idx = np.full(ntot, cfg.SLAB, np.int64)
                idx[1:1 + len(sq)] = sq
                pos = np.cumsum(np.bincount(dq, minlength=cfg.SL))
                idxw[16 * q:16 * (q + 1)] = idx.reshape(-1, 16).T.astype(np.int16)
                posw[16 * q:16 * (q + 1)] = pos.reshape(-1, 16).T.astype(np.int16)
                posp[16 * q:16 * (q + 1)] = pos[None, :].astype(np.int16)
            ins[f"idx{c}"] = idxw
            ins[f"posw{c}"] = posw
            ins[f"posp{c}"] = posp
        per_core.append(ins)
    return meta, per_core


def build(cfg, meta):
    nc = bacc.Bacc("TRN2", target_bir_lowering=False)
    F, H, C = cfg.F_IN, cfg.H, cfg.C
    ND, SL, SLAB, NWIN = cfg.ND, cfg.SL, cfg.SLAB, cfg.NWIN
    SLQ = cfg.SLQ
    def ssize(q):
        return (SLAB - q * SLQ) if q == QC - 1 else min(SLQ, SLAB - q * SLQ)
    TBLN = SLAB + 16
    NTMX = max(meta["ntot"])

    x_t = nc.dram_tensor("x", (F, ND), BF16, kind="ExternalInput")
    w_t = {}
    for nm, sh in [("W1", (F, H)), ("b1", (H,)), ("W2", (H, C)), ("b2", (C,)),
                   ("W3", (C, 32)), ("b3", (32,)), ("W4", (32, C)), ("b4", (C,))]:
        w_t[nm] = nc.dram_tensor(nm, sh, F32, kind="ExternalInput")
    idx_t, posw_t, posp_t = [], [], []
    for c in range(NC):
        idx_t.append(nc.dram_tensor(f"idx{c}", (128, meta["ntot"][c] // 16), I16,
                                    kind="ExternalInput"))
        posw_t.append(nc.dram_tensor(f"posw{c}", (128, SL // 16), I16,
                                     kind="ExternalInput"))
        posp_t.append(nc.dram_tensor(f"posp{c}", (128, SL), I16,
                                     kind="ExternalInput"))
    out_t = nc.dram_tensor("out", (ND, C), F32, kind="ExternalOutput")

    g_loc = [nc.dram_tensor(f"g{i}_loc", (16, SLAB), BF16) for i in (1, 2)]
    g_all = [nc.dram_tensor(f"g{i}_all", (NC, 16, SLAB), BF16,
                            addr_space="Shared") for i in (1, 2)]

    ctx = ExitStack()
    with ctx:
        tc = ctx.enter_context(tile.TileContext(nc))
        P1 = ctx.enter_context(tc.tile_pool(name="P1", bufs=1))
        P2 = ctx.enter_context(tc.tile_pool(name="P2", bufs=2))
        PSA = ctx.enter_context(tc.tile_pool(name="PSA", bufs=2, space="PSUM"))
        PSB = ctx.enter_context(tc.tile_pool(name="PSB", bufs=1, space="PSUM"))

        # ---- constants / weights (before ap_gather library load) ----
        ident = P1.tile([128, 128], F32)
        on1 = P2.tile([128, 128], F32, tag="on1", bufs=1)
        nc.vector.memset(on1[:], 1.0)
        nc.gpsimd.affine_select(ident[:], on1[:], [[1, 128]], AOT.is_equal,
                                0.0, base=0, channel_multiplier=-1)
        ones_sc = P1.tile([128, TILE_E], BF16)
        nc.vector.memset(ones_sc[:], 1.0)

        wW1 = P1.tile([128, cfg.KCH * H], BF16)
        wtmp = P2.tile([128, cfg.KCH * H], F32, tag="wtmp")
        for kb in range(cfg.KCH):
            nc.sync.dma_start(wtmp[:, kb * H:(kb + 1) * H],
                              w_t["W1"][kb * 128:(kb + 1) * 128, :])
        nc.vector.tensor_copy(wW1[:], wtmp[:])
        wsm = {}
        for nm, sh in [("W2", (H, C)), ("W3", (C, 32)), ("W4", (32, C))]:
            wf = P2.tile(list(sh), F32, tag="wtmp")
            nc.sync.dma_start(wf[:], w_t[nm][:])
            wsm[nm] = P1.tile(list(sh), BF16, name=f"wsm_{nm}", tag=f"wsm_{nm}")
            nc.vector.tensor_copy(wsm[nm][:], wf[:])
        vb = {}
        for nm, n in [("b1", H), ("b2", C), ("b3", 32), ("b4", C)]:
            vb[nm] = P1.tile([n, 1], F32, name=f"vb_{nm}", tag=f"vb_{nm}")
            nc.sync.dma_start(vb[nm][:], w_t[nm][:].unsqueeze(1))
        vb1r = P1.tile([128, 1], F32)
        vb2r = P1.tile([128, 1], F32)
        nc.vector.memset(vb1r[:], 0.0)
        nc.vector.memset(vb2r[:], 0.0)
        for q in range(QC):
            nc.sync.dma_start(vb1r[16 * q:16 * q + H, :], vb["b1"][:])
            nc.sync.dma_start(vb2r[16 * q:16 * q + C, :], vb["b2"][:])

        nc.gpsimd.load_library(library_config.ap_gather)

        posw_sb = []
        for c in range(NC):
            pt = P1.tile([128, SL // 16], I16, name=f"poswsb{c}", tag=f"poswsb{c}")
            nc.sync.dma_start(pt[:], posw_t[c][:])
            posw_sb.append(pt)
        NTMX16 = max(meta["ntot"]) // 16

        # ---- deg / dinv (slice layout) ----
        dinv = P1.tile([128, SL], F32)
        degf = P2.tile([128, SL], F32, tag="t0", bufs=1)
        nc.vector.memset(degf[:], 1.0)
        for c in range(NC):
            pp = P2.tile([128, SL], I16, tag="pp", bufs=1)
            nc.sync.dma_start(pp[:], posp_t[c][:])
            tmpp = P2.tile([128, SL], F32, tag="tmpp", bufs=1)
            nc.vector.tensor_copy(tmpp[:], pp[:])
            nc.vector.tensor_tensor(degf[:], degf[:], tmpp[:], AOT.add)
            nc.vector.tensor_tensor(degf[:, 1:SL], degf[:, 1:SL],
                                    tmpp[:, 0:SL - 1], AOT.subtract)
        sqd = P2.tile([128, SL], F32, tag="tmpp", bufs=1)
        nc.scalar.activation(sqd[:], degf[:], AFT.Sqrt)
        nc.vector.reciprocal(dinv[:], sqd[:])

        # ---- conv1 transform -> per-slice fm ministrip -> h1sl ----
        h1sl = P2.tile([128, SL], BF16, tag="hsl")
        NBLK = SL // 128
        with nc.named_scope("conv1mm"):
            for s in range(QC):
                # x arrives host-pre-transposed (feature-major, bf16): load
                # the 4 k-chunks straight into the matmul strip layout.
                xT = P1.tile([128, cfg.KCH * SL], BF16, tag="strip")
                for kb in range(cfg.KCH):
                    eng = (nc.sync, nc.scalar)[kb % 2]
                    eng.dma_start(
                        xT[:, kb * SL:(kb + 1) * SL],
                        x_t[kb * 128:(kb + 1) * 128, s * SL:(s + 1) * SL])
                h1q = P2.tile([16, SL], BF16, tag="fq")
                for n0 in range(0, SL, NWIN):
                    ph = PSA.tile([16, NWIN], F32, tag="ph")
                    for kb in range(cfg.KCH):
                        nc.tensor.matmul(
                            ph[:], wW1[:, kb * H:(kb + 1) * H],
                            xT[:, kb * SL + n0: kb * SL + n0 + NWIN],
                            start=(kb == 0), stop=(kb == cfg.KCH - 1))
                    nc.vector.tensor_copy(h1q[:, n0:n0 + NWIN], ph[:])
                # bridge transform-slice s (nodes [s*SL,(s+1)*SL)) into
                # balanced agg-slices (rank = node - q*SLQ)
                n0, n1 = s * SL, min((s + 1) * SL, SLAB)
                while n0 < n1:
                    q = min(n0 // SLQ, QC - 1)
                    qe = SLAB if q == QC - 1 else min((q + 1) * SLQ, SLAB)
                    pe = min(n1, qe)
                    nc.sync.dma_start(
                        h1sl[16 * q:16 * q + 16, n0 - q * SLQ: pe - q * SLQ],
                        h1q[:, n0 - s * SL: pe - s * SL])
                    n0 = pe

        # ---- aggregation ----
        tbl = P1.tile([128, TBLN], F32, name="tbl")
        nc.vector.memset(tbl[:, SLAB:], 0.0)
        TBH = (SLAB + 1) // 2
        tbs = P1.tile([128, TBH], BF16, name="tbs")
        HALVES = ((0, TBH), (TBH, SLAB))

        def dma_tbs(g_all_dram, c, h):
            # stage one bf16 half of chunk c's table (8 replica DMAs);
            # issued a chunk ahead so the expand-cast sits off the
            # gather critical path.
            lo, hi = HALVES[h]
            for r in range(NC):
                eng = (nc.sync, nc.scalar)[r % 2]
                eng.dma_start(out=tbs[16 * r:16 * r + 16, 0:hi - lo],
                              in_=g_all_dram[c, :, lo:hi])

        def dma_idx(c):
            it = P2.tile([128, NTMX16], I16, tag="idxs")
            nc.sync.dma_start(it[:, 0:meta["ntot"][c] // 16], idx_t[c][:])
            return it

        def cast_half(h):
            lo, hi = HALVES[h]
            nc.vector.tensor_copy(tbl[:, lo:hi], tbs[:, 0:hi - lo])

        def aggregate(g_all_dram, agg_out, scope):
            with nc.named_scope(scope):
                nc.vector.memset(agg_out[:], 0.0)
                idx_next = dma_idx(0)
                dma_tbs(g_all_dram, 0, 0)
                for c in range(NC):
                    ntot = meta["ntot"][c]
                    widths = meta["widths"][c]
                    idxs = idx_next
                    cast_half(0)
                    dma_tbs(g_all_dram, c, 1)
                    cast_half(1)
                    if c + 1 < NC:
                        dma_tbs(g_all_dram, c + 1, 0)
                        idx_next = dma_idx(c + 1)
                    strip = P1.tile([128, NTMX], F32, tag="strip")
                    off = 0
                    for ti, w in enumerate(widths):
                        gout = P2.tile([128, TILE_E], F32, tag="gout")
                        nc.gpsimd.ap_gather(
                            gout[:, 0:w], tbl[:], idxs[:, off // 16:(off + w) // 16],
                            channels=128, num_elems=TBLN, d=1, num_idxs=w)
                        init = 0.0 if ti == 0 else strip[:, off - 1:off]
                        nc.vector.tensor_tensor_scan(
                            strip[:, off:off + w], ones_sc[:, 0:w], gout[:, 0:w],
                            init, AOT.mult, AOT.add)
                        off += w
                    pw = posw_sb[c]
                    samp = P1.tile([128, SL], F32, tag="samp")
                    NPS = min(SL, pad_to(SLQ, 16))
                    if NPS < SL:
                        nc.vector.memset(samp[:, NPS:SL], 0.0)
                    nc.gpsimd.ap_gather(
                        samp[:, 0:NPS], strip[:, 0:ntot], pw[:, 0:NPS // 16],
                        channels=128, num_elems=ntot, d=1, num_idxs=NPS)
                    nc.vector.tensor_tensor(agg_out[:, 0:1], agg_out[:, 0:1],
                                            samp[:, 0:1], AOT.add)
                    dif = P2.tile([128, SL], F32, tag="tmpp", bufs=1)
                    nc.vector.tensor_tensor(dif[:, 1:SL], samp[:, 1:SL],
                                            samp[:, 0:SL - 1], AOT.subtract)
                    nc.vector.tensor_tensor(agg_out[:, 1:SL], agg_out[:, 1:SL],
                                            dif[:, 1:SL], AOT.add)

        def scale_and_ship(hsl, g_loc_d, g_all_d, agname):
            gsl = P2.tile([128, SL], BF16, tag="tmpp", bufs=1)
            nc.vector.tensor_tensor(gsl[:], hsl[:], dinv[:], AOT.mult)
            for q in range(QC):
                wv = ssize(q)
                nc.sync.dma_start(g_loc_d[:, q * SLQ: q * SLQ + wv],
                                  gsl[16 * q:16 * q + 16, 0:wv])
            with nc.named_scope(agname):
                nc.gpsimd.collective_compute(
                    "AllGather", AOT.bypass, replica_groups=[list(range(NC))],
                    ins=[g_loc_d[:]], outs=[g_all_d[:]])

        scale_and_ship(h1sl[:], g_loc[0], g_all[0], "ag1")
        agg = P1.tile([128, SL], F32, tag="agg")
        aggregate(g_all[0], agg[:], "agg1")

        # fin1: a1 = relu(dinv*(agg + dinv*h1) + b1)
        a1sl = P2.tile([128, SL], BF16, tag="hsl")
        t0 = P2.tile([128, SL], F32, tag="t0", bufs=1)
        nc.vector.tensor_tensor(t0[:], h1sl[:], dinv[:], AOT.mult)
        nc.vector.tensor_tensor(t0[:], t0[:], agg[:], AOT.add)
        nc.vector.tensor_tensor(t0[:], t0[:], dinv[:], AOT.mult)
        nc.vector.tensor_scalar(t0[:], t0[:], vb1r[:], None, AOT.add)
        nc.vector.tensor_scalar(a1sl[:], t0[:], 0.0, None, AOT.max)

        # ---- conv2 transform (per q mini-strip) -> h2sl ----
        h2sl = P2.tile([128, SL], BF16, tag="hsl")
        with nc.named_scope("conv2mm"):
            for q in range(QC):
                sq = ssize(q)
                a1q = P2.tile([16, SL], BF16, tag="fq")
                nc.sync.dma_start(a1q[:], a1sl[16 * q:16 * q + 16, :])
                h2q = P2.tile([16, SL], BF16, tag="fq")
                nc.vector.memset(h2q[:], 0.0)
                w0 = 0
                while w0 < sq:
                    wlen = min(NWIN, sq - w0)
                    ph2 = PSB.tile([C, NWIN], F32, tag="ph2")
                    nc.tensor.matmul(
                        ph2[0:C, 0:wlen], wsm["W2"][:],
                        a1q[:, w0:w0 + wlen], start=True, stop=True)
                    nc.vector.tensor_copy(h2q[0:C, w0:w0 + wlen], ph2[0:C, 0:wlen])
                    w0 += wlen
                nc.sync.dma_start(h2sl[16 * q:16 * q + 16, :], h2q[:])

        scale_and_ship(h2sl[:], g_loc[1], g_all[1], "ag2")
        aggregate(g_all[1], agg[:], "agg2")

        # fin2: o2 = dinv*(agg + dinv*h2) + b2  (no relu)
        o2sl = P2.tile([128, SL], F32, tag="t0", bufs=1)
        nc.vector.tensor_tensor(o2sl[:], h2sl[:], dinv[:], AOT.mult)
        nc.vector.tensor_tensor(o2sl[:], o2sl[:], agg[:], AOT.add)
        nc.vector.tensor_tensor(o2sl[:], o2sl[:], dinv[:], AOT.mult)
        nc.vector.tensor_scalar(o2sl[:], o2sl[:], vb2r[:], None, AOT.add)
        o2b = P2.tile([128, SL], BF16, tag="hsl")
        nc.vector.tensor_copy(o2b[:], o2sl[:])

        # ---- head + log_softmax per q ----
        with nc.named_scope("head"):
            for q in range(QC):
                sq = ssize(q)
                nblk = -(-sq // 128)
                o2q = P2.tile([16, SL], BF16, tag="fq")
                nc.sync.dma_start(o2q[:], o2b[16 * q:16 * q + 16, :])
                h4q = P2.tile([C, SL], F32, tag="t0", bufs=1)
                w0 = 0
                while w0 < sq:
                    wlen = min(NWIN, sq - w0)
                    ph3 = PSB.tile([32, NWIN], F32, tag="ph3")
                    nc.tensor.matmul(
                        ph3[0:32, 0:wlen], wsm["W3"][:],
                        o2q[0:C, w0:w0 + wlen], start=True, stop=True)
                    h3w = P2.tile([32, NWIN], BF16, tag="h3w", bufs=1)
                    t3 = P2.tile([32, NWIN], F32, tag="t3", bufs=1)
                    nc.vector.tensor_scalar(t3[:, 0:wlen], ph3[0:32, 0:wlen],
                                            vb["b3"][:], None, AOT.add)
                    nc.vector.tensor_scalar(h3w[:, 0:wlen], t3[:, 0:wlen],
                                            0.0, None, AOT.max)
                    ph4 = PSB.tile([C, NWIN], F32, tag="ph4")
                    nc.tensor.matmul(ph4[0:C, 0:wlen], wsm["W4"][:],
                                     h3w[:, 0:wlen], start=True, stop=True)
                    nc.vector.tensor_scalar(
                        h4q[:, w0:w0 + wlen], ph4[0:C, 0:wlen],
                        vb["b4"][:], None, AOT.add)
                    w0 += wlen
                lstr = P2.tile([128, nblk * C], F32, tag="lstr")
                nc.vector.memset(lstr[:], 0.0)
                for b in range(nblk):
                    blen = min(128, sq - b * 128)
                    ptr = PSB.tile([128, C], F32, tag="ptr")
                    nc.tensor.transpose(ptr[0:blen, :],
                                        h4q[:, b * 128: b * 128 + blen],
                                        ident[0:C, 0:C])
                    nc.vector.tensor_copy(lstr[0:blen, b * C:(b + 1) * C],
                                          ptr[0:blen, :])
                l3 = lstr[:].rearrange("p (b c) -> p b c", c=C)
                mx = P2.tile([128, nblk], F32, tag="mx")
                nc.vector.tensor_reduce(mx[:], l3, axis=mybir.AxisListType.X,
                                        op=AOT.max)
                zt = P2.tile([128, nblk * C], F32, tag="zt")
                z3 = zt[:].rearrange("p (b c) -> p b c", c=C)
                for ci in range(C):
                    nc.vector.tensor_tensor(z3[:, :, ci], l3[:, :, ci], mx[:],
                                            AOT.subtract)
                et = P2.tile([128, nblk * C], F32, tag="t3", bufs=1)
                nc.scalar.activation(et[:], zt[:], AFT.Exp)
                sz = P2.tile([128, nblk], F32, tag="mx")
                nc.vector.tensor_reduce(sz[:], et[:].rearrange("p (b c) -> p b c", c=C),
                                        axis=mybir.AxisListType.X, op=AOT.add)
                lsz = P2.tile([128, nblk], F32, tag="mx")
                nc.scalar.activation(lsz[:], sz[:], AFT.Ln)
                ot = P2.tile([128, nblk * C], F32, tag="zt")
                o3 = ot[:].rearrange("p (b c) -> p b c", c=C)
                for ci in range(C):
                    nc.vector.tensor_tensor(o3[:, :, ci], z3[:, :, ci], lsz[:],
                                            AOT.subtract)
                nfull = sq // 128
                if nfull:
                    nc.sync.dma_start(
                        out_t[q * SLQ: q * SLQ + nfull * 128, :].rearrange(
                            "(b p) c -> p b c", p=128),
                        ot[:, 0:nfull * C].rearrange("p (b c) -> p b c", c=C))
                tail = sq - nfull * 128
                if tail:
                    nc.sync.dma_start(
                        out_t[q * SLQ + nfull * 128: q * SLQ + sq, :],
                        ot[0:tail, nfull * C:(nfull + 1) * C])

    nc.compile()
    return nc


# ---------------------------------------------------------------------------
# host-side sharding + compile cache + runner
# ---------------------------------------------------------------------------
N_NODES, F_IN_C, H_C, C_C = 100000, 512, 16, 5
_cache = {}

last_exec_time_ns = None


def kernel(x, edge_index, W1, b1, W2, b2, W3, b3, W4, b4):
    global last_exec_time_ns
    from concourse import bass_utils

    x = np.ascontiguousarray(np.asarray(x, np.float32))
    edge_index = np.asarray(edge_index)
    assert x.shape == (N_NODES, F_IN_C), x.shape
    cfg = Cfg(N_NODES, F_IN_C, H_C, C_C)
    meta, per_core = host_prep(edge_index, cfg)
    key = tuple(meta["ntot"])
    if key not in _cache:
        _cache[key] = build(cfg, meta)
    nc = _cache[key]

    wts = dict(W1=np.asarray(W1, np.float32), b1=np.asarray(b1, np.float32),
               W2=np.asarray(W2, np.float32), b2=np.asarray(b2, np.float32),
               W3=np.asarray(W3, np.float32), b3=np.asarray(b3, np.float32),
               W4=np.asarray(W4, np.float32), b4=np.asarray(b4, np.float32))
    import ml_dtypes
    in_maps = []
    for k in range(NC):
        lo = k * cfg.SLAB
        hi = min((k + 1) * cfg.SLAB, N_NODES)
        xs = np.zeros((F_IN_C, cfg.ND), ml_dtypes.bfloat16)
        xs[:, :hi - lo] = x[lo:hi].astype(ml_dtypes.bfloat16).T
        im = dict(x=xs, **wts)
        im.update(per_core[k])
        in_maps.append(im)

    res = bass_utils.run_bass_kernel_spmd(
        nc, in_maps, list(range(NC)), trace=_HOOK_OK,
        trace_cores=[0] if _HOOK_OK else None)
    last_exec_time_ns = res.exec_time_ns

    outs = []
    for k in range(NC):
        lo = k * cfg.SLAB
        hi = min((k + 1) * cfg.SLAB, N_NODES)
        outs.append(res.results[k]["out"][:hi - lo])
    return np.ascontiguousarray(np.concatenate(outs).astype(np.float32))



# revision 43
# speedup vs baseline: 1.0181x; 1.0034x over previous
"""Self-contained Trainium2 Bass kernel for the 2-layer GCN problem.

kernel(**inputs) -> np.ndarray [100000, 5] float32 (log-softmax outputs).

Strategy: destination-node slabs of 12500 across the 8 NeuronCores.
x arrives host-pre-transposed to feature-major bf16 (F, ND) so conv1
loads straight into the PE strip layout; activations ship through the
AllGather as bf16 and are expanded to the fp32 gather table on the DVE
via a half-slab staging tile.  Graph aggregation per source-slab chunk:
GPSIMD ap_gather from the table (replicated over the 8 Q7 core groups),
chained fp32 running cumsum on the DVE, one boundary ap_gather at
per-node last-edge positions, adjacent diff.  Degrees are derived
on-device from the dst-segment position tensors.  The host does index
preprocessing (sorting/partitioning edge_index per the sharding
contract), the x transpose/cast, and input sharding/unsharding.
"""
import os, sys
sys.path.insert(0, "/opt/trn_rl_repo")

_HOOK_OK = False


def _install_ntff_hook():
    global _HOOK_OK
    try:
        import types
        import antenv
        mod = types.ModuleType("antenv.axon_hooks")
        _h = {}
        mod.set_axon_ntff_profile_hook = lambda h: _h.__setitem__("h", h)
        mod.get_axon_ntff_profile_hook = lambda: _h.get("h")
        sys.modules["antenv.axon_hooks"] = mod
        antenv.axon_hooks = mod
        from trn_agent_boot.trn_boot import _ntff_profile_via_ctypes
        mod.set_axon_ntff_profile_hook(
            _ntff_profile_via_ctypes("/opt/axon/libaxon_pjrt.so"))
        from concourse import bass_utils as _bu
        _bu.upload_artifacts = lambda tmpdir: tmpdir
        _HOOK_OK = True
    except Exception:
        _HOOK_OK = False


_install_ntff_hook()

"""builder"""
import numpy as np
from contextlib import ExitStack

import concourse.bacc as bacc
import concourse.bass as bass
import concourse.mybir as mybir
import concourse.tile as tile
from concourse import library_config

F32 = mybir.dt.float32
BF16 = mybir.dt.bfloat16
I16 = mybir.dt.int16
AOT = mybir.AluOpType
AFT = mybir.ActivationFunctionType

NC = 8
QC = 8
TILE_E = 2560


def pad_to(x, m):
    return (x + m - 1) // m * m


class Cfg:
    def __init__(self, n_nodes, f_in=512, h=16, c=5):
        self.N = n_nodes
        self.SLAB = -(-n_nodes // NC)
        self.SL = pad_to(-(-self.SLAB // QC), 128)
        self.ND = self.SL * QC
        self.SLQ = -(-self.SLAB // QC)         # balanced agg-slice size
        self.F_IN, self.H, self.C = f_in, h, c
        self.KCH = f_in // 128
        nwin = None
        for cand in (512, 448, 416, 384, 320, 256, 128):
            if self.SL % cand == 0:
                nwin = cand
                break
        assert nwin, self.SL
        self.NWIN = nwin


def host_prep(edge_index, cfg):
    src = np.asarray(edge_index[0], np.int64)
    dst = np.asarray(edge_index[1], np.int64)
    cores = []
    for k in range(NC):
        d_lo = k * cfg.SLAB
        d_hi = min((k + 1) * cfg.SLAB, cfg.N)
        sel = (dst >= d_lo) & (dst < d_hi)
        s, d = src[sel], dst[sel] - d_lo
        chunks = []
        for c in range(NC):
            lo = c * cfg.SLAB
            hi = min(lo + cfg.SLAB, cfg.N)
            m = (s >= lo) & (s < hi)
            sc, dc = s[m] - lo, d[m]
            o = np.argsort(dc, kind="stable")
            chunks.append((sc[o].astype(np.int64), dc[o].astype(np.int64)))
        cores.append(chunks)
    meta = dict(ntot=[], widths=[])
    for c in range(NC):
        maxe = 1
        for k in range(NC):
            sc, dc = cores[k][c]
            if len(dc):
                maxe = max(maxe, int(np.bincount(
                    np.minimum(dc // cfg.SLQ, QC - 1), minlength=QC).max()))
        ntot = pad_to(maxe + 1, 16)
        # split into ~equal tiles of at most TILE_E; non-final widths must be
        # %64 so every tile's idx-slice starts 4B-aligned (the Q7 idx
        # preload reads the int16 stream as 32-bit words).
        nt = -(-ntot // TILE_E)
        widths = []
        rem = ntot
        for t in range(nt, 0, -1):
            w = min(pad_to(-(-rem // t), 64), TILE_E) if t > 1 else rem
            w = min(w, rem)
            widths.append(w)
            rem -= w
        assert sum(widths) == ntot and all(x % 16 == 0 for x in widths)
        assert all(x % 64 == 0 for x in widths[:-1])
        meta["ntot"].append(ntot)
        meta["widths"].append(widths)
    per_core = []
    for k in range(NC):
        ins = {}
        for c in range(NC):
            ntot = meta["ntot"][c]
            sc, dc = cores[k][c]
            idxw = np.zeros((128, ntot // 16), np.int16)
            posw = np.zeros((128, cfg.SL // 16), np.int16)
            posp = np.zeros((128, cfg.SL), np.int16)
            for q in range(QC):
                a = q * cfg.SLQ
                b = min((q + 1) * cfg.SLQ, cfg.SLAB) if q < QC - 1 else cfg.SLAB
                mm = (dc >= a) & (dc < b)
                sq = sc[mm]
                dq = dc[mm] - a
                idx = np.full(ntot, cfg.SLAB, np.int64)
                idx[1:1 + len(sq)] = sq
                pos = np.cumsum(np.bincount(dq, minlength=cfg.SL))
                idxw[16 * q:16 * (q + 1)] = idx.reshape(-1, 16).T.astype(np.int16)
                posw[16 * q:16 * (q + 1)] = pos.reshape(-1, 16).T.astype(np.int16)
                posp[16 * q:16 * (q + 1)] = pos[None, :].astype(np.int16)
            ins[f"idx{c}"] = idxw
            ins[f"posw{c}"] = posw
            ins[f"posp{c}"] = posp
        per_core.append(ins)
    return meta, per_core


def build(cfg, meta):
    nc = bacc.Bacc("TRN2", target_bir_lowering=False)
    F, H, C = cfg.F_IN, cfg.H, cfg.C
    ND, SL, SLAB, NWIN = cfg.ND, cfg.SL, cfg.SLAB, cfg.NWIN
    SLQ = cfg.SLQ
    def ssize(q):
        return (SLAB - q * SLQ) if q == QC - 1 else min(SLQ, SLAB - q * SLQ)
    TBLN = SLAB + 16
    NTMX = max(meta["ntot"])

    x_t = nc.dram_tensor("x", (F, ND), BF16, kind="ExternalInput")
    w_t = {}
    for nm, sh in [("W1", (F, H)), ("b1", (H,)), ("W2", (H, C)), ("b2", (C,)),
                   ("W3", (C, 32)), ("b3", (32,)), ("W4", (32, C)), ("b4", (C,))]:
        w_t[nm] = nc.dram_tensor(nm, sh, F32, kind="ExternalInput")
    idx_t, posw_t, posp_t = [], [], []
    for c in range(NC):
        idx_t.append(nc.dram_tensor(f"idx{c}", (128, meta["ntot"][c] // 16), I16,
                                    kind="ExternalInput"))
        posw_t.append(nc.dram_tensor(f"posw{c}", (128, SL // 16), I16,
                                     kind="ExternalInput"))
        posp_t.append(nc.dram_tensor(f"posp{c}", (128, SL), I16,
                                     kind="ExternalInput"))
    out_t = nc.dram_tensor("out", (ND, C), F32, kind="ExternalOutput")

    g_loc = [nc.dram_tensor(f"g{i}_loc", (16, SLAB), BF16) for i in (1, 2)]
    g_all = [nc.dram_tensor(f"g{i}_all", (NC, 16, SLAB), BF16,
                            addr_space="Shared") for i in (1, 2)]

    ctx = ExitStack()
    with ctx:
        tc = ctx.enter_context(tile.TileContext(nc))
        P1 = ctx.enter_context(tc.tile_pool(name="P1", bufs=1))
        P2 = ctx.enter_context(tc.tile_pool(name="P2", bufs=2))
        PSA = ctx.enter_context(tc.tile_pool(name="PSA", bufs=2, space="PSUM"))
        PSB = ctx.enter_context(tc.tile_pool(name="PSB", bufs=1, space="PSUM"))

        # ---- constants / weights (before ap_gather library load) ----
        ident = P1.tile([128, 128], F32)
        on1 = P2.tile([128, 128], F32, tag="on1", bufs=1)
        nc.vector.memset(on1[:], 1.0)
        nc.gpsimd.affine_select(ident[:], on1[:], [[1, 128]], AOT.is_equal,
                                0.0, base=0, channel_multiplier=-1)
        ones_sc = P1.tile([128, TILE_E], BF16)
        nc.vector.memset(ones_sc[:], 1.0)

        wW1 = P1.tile([128, cfg.KCH * H], BF16)
        wtmp = P2.tile([128, cfg.KCH * H], F32, tag="wtmp")
        for kb in range(cfg.KCH):
            nc.sync.dma_start(wtmp[:, kb * H:(kb + 1) * H],
                              w_t["W1"][kb * 128:(kb + 1) * 128, :])
        nc.vector.tensor_copy(wW1[:], wtmp[:])
        wsm = {}
        for nm, sh in [("W2", (H, C)), ("W3", (C, 32)), ("W4", (32, C))]:
            wf = P2.tile(list(sh), F32, tag="wtmp")
            nc.sync.dma_start(wf[:], w_t[nm][:])
            wsm[nm] = P1.tile(list(sh), BF16, name=f"wsm_{nm}", tag=f"wsm_{nm}")
            nc.vector.tensor_copy(wsm[nm][:], wf[:])
        vb = {}
        for nm, n in [("b1", H), ("b2", C), ("b3", 32), ("b4", C)]:
            vb[nm] = P1.tile([n, 1], F32, name=f"vb_{nm}", tag=f"vb_{nm}")
            nc.sync.dma_start(vb[nm][:], w_t[nm][:].unsqueeze(1))
        vb1r = P1.tile([128, 1], F32)
        vb2r = P1.tile([128, 1], F32)
        nc.vector.memset(vb1r[:], 0.0)
        nc.vector.memset(vb2r[:], 0.0)
        for q in range(QC):
            nc.sync.dma_start(vb1r[16 * q:16 * q + H, :], vb["b1"][:])
            nc.sync.dma_start(vb2r[16 * q:16 * q + C, :], vb["b2"][:])

        nc.gpsimd.load_library(library_config.ap_gather)

        posw_sb = []
        for c in range(NC):
            pt = P1.tile([128, SL // 16], I16, name=f"poswsb{c}", tag=f"poswsb{c}")
            nc.sync.dma_start(pt[:], posw_t[c][:])
            posw_sb.append(pt)
        NTMX16 = max(meta["ntot"]) // 16

        # ---- deg / dinv (slice layout) ----
        dinv = P1.tile([128, SL], F32)
        degf = P2.tile([128, SL], F32, tag="t0", bufs=1)
        nc.vector.memset(degf[:], 1.0)
        for c in range(NC):
            pp = P2.tile([128, SL], I16, tag="pp", bufs=1)
            nc.sync.dma_start(pp[:], posp_t[c][:])
            tmpp = P2.tile([128, SL], F32, tag="tmpp", bufs=1)
            nc.vector.tensor_copy(tmpp[:], pp[:])
            nc.vector.tensor_tensor(degf[:], degf[:], tmpp[:], AOT.add)
            nc.vector.tensor_tensor(degf[:, 1:SL], degf[:, 1:SL],
                                    tmpp[:, 0:SL - 1], AOT.subtract)
        sqd = P2.tile([128, SL], F32, tag="tmpp", bufs=1)
        nc.scalar.activation(sqd[:], degf[:], AFT.Sqrt)
        nc.vector.reciprocal(dinv[:], sqd[:])

        # ---- conv1 transform -> per-slice fm ministrip -> h1sl ----
        h1sl = P2.tile([128, SL], BF16, tag="hsl")
        NBLK = SL // 128
        with nc.named_scope("conv1mm"):
            for s in range(QC):
                # x arrives host-pre-transposed (feature-major, bf16): load
                # the 4 k-chunks straight into the matmul strip layout.
                xT = P1.tile([128, cfg.KCH * SL], BF16, tag="strip")
                for kb in range(cfg.KCH):
                    eng = (nc.sync, nc.scalar)[kb % 2]
                    eng.dma_start(
                        xT[:, kb * SL:(kb + 1) * SL],
                        x_t[kb * 128:(kb + 1) * 128, s * SL:(s + 1) * SL])
                h1q = P2.tile([16, SL], BF16, tag="fq")
                for n0 in range(0, SL, NWIN):
                    ph = PSA.tile([16, NWIN], F32, tag="ph")
                    for kb in range(cfg.KCH):
                        nc.tensor.matmul(
                            ph[:], wW1[:, kb * H:(kb + 1) * H],
                            xT[:, kb * SL + n0: kb * SL + n0 + NWIN],
                            start=(kb == 0), stop=(kb == cfg.KCH - 1))
                    nc.vector.tensor_copy(h1q[:, n0:n0 + NWIN], ph[:])
                # bridge transform-slice s (nodes [s*SL,(s+1)*SL)) into
                # balanced agg-slices (rank = node - q*SLQ)
                n0, n1 = s * SL, min((s + 1) * SL, SLAB)
                while n0 < n1:
                    q = min(n0 // SLQ, QC - 1)
                    qe = SLAB if q == QC - 1 else min((q + 1) * SLQ, SLAB)
                    pe = min(n1, qe)
                    nc.sync.dma_start(
                        h1sl[16 * q:16 * q + 16, n0 - q * SLQ: pe - q * SLQ],
                        h1q[:, n0 - s * SL: pe - s * SL])
                    n0 = pe

        # ---- aggregation ----
        tbl = P1.tile([128, TBLN], F32, name="tbl")
        nc.vector.memset(tbl[:, SLAB:], 0.0)
        TBH = (SLAB + 1) // 2
        tbs = P1.tile([128, TBH], BF16, name="tbs")
        HALVES = ((0, TBH), (TBH, SLAB))

        def dma_tbs(g_all_dram, c, h):
            # stage one bf16 half of chunk c's table (8 replica DMAs);
            # issued a chunk ahead so the expand-cast sits off the
            # gather critical path.
            lo, hi = HALVES[h]
            for r in range(NC):
                eng = (nc.sync, nc.scalar)[r % 2]
                eng.dma_start(out=tbs[16 * r:16 * r + 16, 0:hi - lo],
                              in_=g_all_dram[c, :, lo:hi])

        def dma_idx(c):
            it = P2.tile([128, NTMX16], I16, tag="idxs")
            nc.sync.dma_start(it[:, 0:meta["ntot"][c] // 16], idx_t[c][:])
            return it

        def cast_half(h):
            lo, hi = HALVES[h]
            nc.vector.tensor_copy(tbl[:, lo:hi], tbs[:, 0:hi - lo])

        def aggregate(g_all_dram, agg_out, scope):
            with nc.named_scope(scope):
                nc.vector.memset(agg_out[:], 0.0)
                idx_next = dma_idx(0)
                dma_tbs(g_all_dram, 0, 0)
                for c in range(NC):
                    ntot = meta["ntot"][c]
                    widths = meta["widths"][c]
                    idxs = idx_next
                    cast_half(0)
                    dma_tbs(g_all_dram, c, 1)
                    cast_half(1)
                    if c + 1 < NC:
                        dma_tbs(g_all_dram, c + 1, 0)
                        idx_next = dma_idx(c + 1)
                    strip = P1.tile([128, NTMX], F32, tag="strip")
                    off = 0
                    for ti, w in enumerate(widths):
                        gout = P2.tile([128, TILE_E], F32, tag="gout")
                        nc.gpsimd.ap_gather(
                            gout[:, 0:w], tbl[:], idxs[:, off // 16:(off + w) // 16],
                            channels=128, num_elems=TBLN, d=1, num_idxs=w)
                        init = 0.0 if ti == 0 else strip[:, off - 1:off]
                        nc.vector.tensor_tensor_scan(
                            strip[:, off:off + w], ones_sc[:, 0:w], gout[:, 0:w],
                            init, AOT.mult, AOT.add)
                        off += w
                    pw = posw_sb[c]
                    samp = P1.tile([128, SL], F32, tag="samp")
                    NPS = min(SL, pad_to(SLQ, 16))
                    if NPS < SL:
                        nc.vector.memset(samp[:, NPS:SL], 0.0)
                    nc.gpsimd.ap_gather(
                        samp[:, 0:NPS], strip[:, 0:ntot], pw[:, 0:NPS // 16],
                        channels=128, num_elems=ntot, d=1, num_idxs=NPS)
                    nc.vector.tensor_tensor(agg_out[:, 0:1], agg_out[:, 0:1],
                                            samp[:, 0:1], AOT.add)
                    dif = P2.tile([128, SL], F32, tag="tmpp", bufs=1)
                    nc.vector.tensor_tensor(dif[:, 1:SL], samp[:, 1:SL],
                                            samp[:, 0:SL - 1], AOT.subtract)
                    nc.vector.tensor_tensor(agg_out[:, 1:SL], agg_out[:, 1:SL],
                                            dif[:, 1:SL], AOT.add)

        def scale_and_ship(hsl, g_loc_d, g_all_d, agname):
            gsl = P2.tile([128, SL], BF16, tag="tmpp", bufs=1)
            nc.vector.tensor_tensor(gsl[:], hsl[:], dinv[:], AOT.mult)
            for q in range(QC):
                wv = ssize(q)
                nc.sync.dma_start(g_loc_d[:, q * SLQ: q * SLQ + wv],
                                  gsl[16 * q:16 * q + 16, 0:wv])
            with nc.named_scope(agname):
                nc.gpsimd.collective_compute(
                    "AllGather", AOT.bypass, replica_groups=[list(range(NC))],
                    ins=[g_loc_d[:]], outs=[g_all_d[:]])

        scale_and_ship(h1sl[:], g_loc[0], g_all[0], "ag1")
        agg = P1.tile([128, SL], F32, tag="agg")
        aggregate(g_all[0], agg[:], "agg1")

        # fin1: a1 = relu(dinv*(agg + dinv*h1) + b1)
        a1sl = P2.tile([128, SL], BF16, tag="hsl")
        t0 = P2.tile([128, SL], F32, tag="t0", bufs=1)
        nc.vector.tensor_tensor(t0[:], h1sl[:], dinv[:], AOT.mult)
        nc.vector.tensor_tensor(t0[:], t0[:], agg[:], AOT.add)
        nc.vector.tensor_tensor(t0[:], t0[:], dinv[:], AOT.mult)
        nc.vector.tensor_scalar(t0[:], t0[:], vb1r[:], None, AOT.add)
        nc.vector.tensor_scalar(a1sl[:], t0[:], 0.0, None, AOT.max)

        # ---- conv2 transform (per q mini-strip) -> h2sl ----
        h2sl = P2.tile([128, SL], BF16, tag="hsl")
        with nc.named_scope("conv2mm"):
            for q in range(QC):
                sq = ssize(q)
                a1q = P2.tile([16, SL], BF16, tag="fq")
                nc.sync.dma_start(a1q[:], a1sl[16 * q:16 * q + 16, :])
                h2q = P2.tile([16, SL], BF16, tag="fq")
                nc.vector.memset(h2q[:], 0.0)
                w0 = 0
                while w0 < sq:
                    wlen = min(NWIN, sq - w0)
                    ph2 = PSB.tile([C, NWIN], F32, tag="ph2")
                    nc.tensor.matmul(
                        ph2[0:C, 0:wlen], wsm["W2"][:],
                        a1q[:, w0:w0 + wlen], start=True, stop=True)
                    nc.vector.tensor_copy(h2q[0:C, w0:w0 + wlen], ph2[0:C, 0:wlen])
                    w0 += wlen
                nc.sync.dma_start(h2sl[16 * q:16 * q + 16, :], h2q[:])

        scale_and_ship(h2sl[:], g_loc[1], g_all[1], "ag2")
        aggregate(g_all[1], agg[:], "agg2")

        # fin2: o2 = dinv*(agg + dinv*h2) + b2  (no relu)
        o2sl = P2.tile([128, SL], F32, tag="t0", bufs=1)
        nc.vector.tensor_tensor(o2sl[:], h2sl[:], dinv[:], AOT.mult)
        nc.vector.tensor_tensor(o2sl[:], o2sl[:], agg[:], AOT.add)
        nc.vector.tensor_tensor(o2sl[:], o2sl[:], dinv[:], AOT.mult)
        nc.vector.tensor_scalar(o2sl[:], o2sl[:], vb2r[:], None, AOT.add)
        o2b = P2.tile([128, SL], BF16, tag="hsl")
        nc.vector.tensor_copy(o2b[:], o2sl[:])

        # ---- head + log_softmax per q ----
        with nc.named_scope("head"):
            for q in range(QC):
                sq = ssize(q)
                nblk = -(-sq // 128)
                o2q = P2.tile([16, SL], BF16, tag="fq")
                nc.sync.dma_start(o2q[:], o2b[16 * q:16 * q + 16, :])
                h4q = P2.tile([C, SL], F32, tag="t0", bufs=1)
                w0 = 0
                while w0 < sq:
                    wlen = min(NWIN, sq - w0)
                    ph3 = PSB.tile([32, NWIN], F32, tag="ph3")
                    nc.tensor.matmul(
                        ph3[0:32, 0:wlen], wsm["W3"][:],
                        o2q[0:C, w0:w0 + wlen], start=True, stop=True)
                    h3w = P2.tile([32, NWIN], BF16, tag="h3w", bufs=1)
                    t3 = P2.tile([32, NWIN], F32, tag="t3", bufs=1)
                    nc.vector.tensor_scalar(t3[:, 0:wlen], ph3[0:32, 0:wlen],
                                            vb["b3"][:], None, AOT.add)
                    nc.vector.tensor_scalar(h3w[:, 0:wlen], t3[:, 0:wlen],
                                            0.0, None, AOT.max)
                    ph4 = PSB.tile([C, NWIN], F32, tag="ph4")
                    nc.tensor.matmul(ph4[0:C, 0:wlen], wsm["W4"][:],
                                     h3w[:, 0:wlen], start=True, stop=True)
                    nc.vector.tensor_scalar(
                        h4q[:, w0:w0 + wlen], ph4[0:C, 0:wlen],
                        vb["b4"][:], None, AOT.add)
                    w0 += wlen
                lstr = P2.tile([128, nblk * C], F32, tag="lstr")
                nc.vector.memset(lstr[:], 0.0)
                for b in range(nblk):
                    blen = min(128, sq - b * 128)
                    ptr = PSB.tile([128, C], F32, tag="ptr")
                    nc.tensor.transpose(ptr[0:blen, :],
                                        h4q[:, b * 128: b * 128 + blen],
                                        ident[0:C, 0:C])
                    nc.vector.tensor_copy(lstr[0:blen, b * C:(b + 1) * C],
                                          ptr[0:blen, :])
                l3 = lstr[:].rearrange("p (b c) -> p b c", c=C)
                mx = P2.tile([128, nblk], F32, tag="mx")
                nc.vector.tensor_reduce(mx[:], l3, axis=mybir.AxisListType.X,
                                        op=AOT.max)
                zt = P2.tile([128, nblk * C], F32, tag="zt")
                z3 = zt[:].rearrange("p (b c) -> p b c", c=C)
                for ci in range(C):
                    nc.vector.tensor_tensor(z3[:, :, ci], l3[:, :, ci], mx[:],
                                            AOT.subtract)
                et = P2.tile([128, nblk * C], F32, tag="t3", bufs=1)
                nc.scalar.activation(et[:], zt[:], AFT.Exp)
                sz = P2.tile([128, nblk], F32, tag="mx")
                nc.vector.tensor_reduce(sz[:], et[:].rearrange("p (b c) -> p b c", c=C),
                                        axis=mybir.AxisListType.X, op=AOT.add)
                lsz = P2.tile([128, nblk], F32, tag="mx")
                nc.scalar.activation(lsz[:], sz[:], AFT.Ln)
                ot = P2.tile([128, nblk * C], F32, tag="zt")
                o3 = ot[:].rearrange("p (b c) -> p b c", c=C)
                for ci in range(C):
                    nc.vector.tensor_tensor(o3[:, :, ci], z3[:, :, ci], lsz[:],
                                            AOT.subtract)
                nfull = sq // 128
                if nfull:
                    nc.sync.dma_start(
                        out_t[q * SLQ: q * SLQ + nfull * 128, :].rearrange(
                            "(b p) c -> p b c", p=128),
                        ot[:, 0:nfull * C].rearrange("p (b c) -> p b c", c=C))
                tail = sq - nfull * 128
                if tail:
                    nc.sync.dma_start(
                        out_t[q * SLQ + nfull * 128: q * SLQ + sq, :],
                        ot[0:tail, nfull * C:(nfull + 1) * C])

    nc.compile()
    return nc


# ---------------------------------------------------------------------------
# host-side sharding + compile cache + runner
# ---------------------------------------------------------------------------
N_NODES, F_IN_C, H_C, C_C = 100000, 512, 16, 5
_cache = {}

last_exec_time_ns = None


def kernel(x, edge_index, W1, b1, W2, b2, W3, b3, W4, b4):
    global last_exec_time_ns
    from concourse import bass_utils

    x = np.ascontiguousarray(np.asarray(x, np.float32))
    edge_index = np.asarray(edge_index)
    assert x.shape == (N_NODES, F_IN_C), x.shape
    cfg = Cfg(N_NODES, F_IN_C, H_C, C_C)
    meta, per_core = host_prep(edge_index, cfg)
    key = tuple(meta["ntot"])
    if key not in _cache:
        _cache[key] = build(cfg, meta)
    nc = _cache[key]

    wts = dict(W1=np.asarray(W1, np.float32), b1=np.asarray(b1, np.float32),
               W2=np.asarray(W2, np.float32), b2=np.asarray(b2, np.float32),
               W3=np.asarray(W3, np.float32), b3=np.asarray(b3, np.float32),
               W4=np.asarray(W4, np.float32), b4=np.asarray(b4, np.float32))
    import ml_dtypes
    in_maps = []
    for k in range(NC):
        lo = k * cfg.SLAB
        hi = min((k + 1) * cfg.SLAB, N_NODES)
        xs = np.zeros((F_IN_C, cfg.ND), ml_dtypes.bfloat16)
        xs[:, :hi - lo] = x[lo:hi].astype(ml_dtypes.bfloat16).T
        im = dict(x=xs, **wts)
        im.update(per_core[k])
        in_maps.append(im)

    res = bass_utils.run_bass_kernel_spmd(
        nc, in_maps, list(range(NC)), trace=_HOOK_OK,
        trace_cores=[0] if _HOOK_OK else None)
    last_exec_time_ns = res.exec_time_ns

    outs = []
    for k in range(NC):
        lo = k * cfg.SLAB
        hi = min((k + 1) * cfg.SLAB, N_NODES)
        outs.append(res.results[k]["out"][:hi - lo])
    return np.ascontiguousarray(np.concatenate(outs).astype(np.float32))

